# revision 1
# baseline (speedup 1.0000x reference)
"""GAT (2-layer PyG GATConv, eval) on 8 Trainium2 NeuronCores.

Sharding: nodes range-partitioned (NLOC=12800/core); each core owns edges whose
dst is in its range, grouped by (128-dst window, src quadrant) into static
640-slot groups. Per edge: [h|alpha_src] rows come from dma_gather (int16 idx
into 25600-row sub-tables), alpha_dst from a 32B-payload dma_gather on a local
table. Softmax uses the shift-invariant no-max form (|e| < ~25, fp32 exp safe):
w = exp(leaky_relu(as+ad)); out = sum(w h)/sum(w). Segment reduction is a PE
matmul with a one-hot(dst_rel) stationary matrix accumulating dst-major
[128, C] PSUM per window — no scatter instructions (HW scatter-add races on
duplicate indices). One transposed-fp16 AllGather links the layers.
"""
import numpy as np
import ml_dtypes

N = 100000
E = 1600000
NF = 256
HEADS, NHID = 8, 8
NH = HEADS * NHID          # 64
NCLASS = 40
NLOC = 12800               # nodes per core
NW = 100                   # 128-dst windows per core
NQ = 4                     # src quadrants
QS = 25600                 # sub-table rows per quadrant
SQ = 640                   # edge slots per (window, quadrant) group
CPG = SQ // 128            # 5 chunks per group
CPW = CPG * NQ             # 20 chunks per window
NSLOT = NW * NQ * SQ       # 256000 slots per core
NTOT = 102400
NCHK = 100352 // 128       # 784 node chunks in the global pass
ACC_EPS = 1e-16

_CACHE = {}


def _host_prep(x, edge_index, W1, a1_src, a1_dst, b1, W2, a2_src, a2_dst, b2):
    src = np.asarray(edge_index[0], dtype=np.int64)
    dst = np.asarray(edge_index[1], dtype=np.int64)

    core = dst // NLOC
    dloc = dst - core * NLOC
    win = dloc >> 7
    q = src // QS
    gid = ((core * NW) + win) * NQ + q
    order = np.argsort(gid, kind="stable")
    gsz = np.bincount(gid, minlength=8 * NW * NQ)
    assert gsz.max() <= SQ, f"group overflow: {gsz.max()} > {SQ}"
    starts = np.zeros_like(gsz)
    starts[1:] = np.cumsum(gsz)[:-1]
    g_sorted = gid[order]
    rank = np.arange(E) - starts[g_sorted]
    slot_global = g_sorted * SQ + rank
    cc = slot_global // NSLOT
    sc = slot_global - cc * NSLOT

    hidx = np.zeros((8, NSLOT), np.int16)          # pad -> row 0 of sub-table
    drel = np.full((8, NSLOT), 128.0, np.float32)  # pad -> out-of-window
    hidx[cc, sc] = (src[order] - q[order] * QS).astype(np.int16)
    drel[cc, sc] = (dloc[order] & 127).astype(np.float32)

    # dma_gather idx wrap: token s -> [s%16, s//16], replicated into all eight
    # 16-partition blocks (Q7 cpu pairs for the 4 SWDGE queues).
    slots = np.arange(NSLOT)
    hw = np.zeros((8, 128, NSLOT // 16), np.int16)
    hw[:, slots % 16, slots // 16] = hidx
    for r in range(1, 8):
        hw[:, 16 * r:16 * (r + 1)] = hw[:, :16]

    # dst_rel, chunk-major: token s -> [s%128, s//128]
    dw = np.zeros((8, 128, NSLOT // 128), np.float32)
    dw[:, slots % 128, slots // 128] = drel
    dw = dw.astype(ml_dtypes.bfloat16)

    # fold attention vectors into the node-pass weights
    W1 = np.asarray(W1, np.float32)
    v_s1 = np.einsum("chk,hk->ch", W1.reshape(NF, HEADS, NHID),
                     np.asarray(a1_src, np.float32))
    v_d1 = np.einsum("chk,hk->ch", W1.reshape(NF, HEADS, NHID),
                     np.asarray(a1_dst, np.float32))
    W1e = np.concatenate([W1, v_s1, v_d1], axis=1).reshape(2, 128, 80)
    W1e = W1e.astype(np.float16)

    W2 = np.asarray(W2, np.float32)
    v_s2 = W2 @ np.asarray(a2_src, np.float32)[0]
    v_d2 = W2 @ np.asarray(a2_dst, np.float32)[0]
    W2e = np.concatenate([W2, v_s2[:, None]], axis=1).astype(np.float16)

    xp = np.zeros((NTOT, NF), np.float32)
    xp[:N] = np.asarray(x, np.float32)
    xT = np.ascontiguousarray(xp[:100352].T).astype(np.float16)
    xT = xT.reshape(2, 128, 100352)

    per_core = []
    for c in range(8):
        xloc = np.ascontiguousarray(xp[c * NLOC:(c + 1) * NLOC].T)
        per_core.append({
            "xT": xT,
            "xTloc": xloc.astype(np.float16).reshape(2, 128, NLOC),
            "W1e": W1e,
            "W2e": W2e,
            "vd2": v_d2[None, :].astype(np.float16),
            "b1": np.asarray(b1, np.float32)[None, :],
            "b2": np.asarray(b2, np.float32)[None, :],
            "hidx": hw[c],
            "drel": dw[c],
        })
    return per_core


def _dma_gather_small(g, out_ap, in_ap, idxs_ap, num_idxs, elem_size,
                      elem_step, queue_num=0):
    """dma_gather with a <256B payload. Only the row STRIDE must be a 256B
    multiple on the Q7 side; bass's elem_size%256 assert is over-conservative
    for the non-transpose path, so build the instruction directly."""
    import concourse.mybir as mybir
    stride_bytes = elem_step * mybir.dt.size(in_ap.dtype)
    assert stride_bytes % 256 == 0
    _in_ap = g.lower_ap_dma(in_ap, for_custom_bir_dma=True)
    _idxs_ap = g.lower_ap(idxs_ap)
    _out_ap = g.lower_ap(out_ap)
    return g.add_instruction(mybir.InstDMAGatherAnt(
        name=g.bass.get_next_instruction_name(),
        ins=[*_in_ap, _idxs_ap, g.lower_val_access(g.to_reg(num_idxs))],
        outs=[_out_ap],
        transpose=False,
        num_idxs=num_idxs,
        elem_size=elem_size,
        stride_bytes_256=stride_bytes // 256,
        gen_mode=0,
        single_packet=True,
        queue_num=queue_num,
        sbuf_tokens_per_rank=0,
        sbuf_free_dim_per_rank=0,
        sbuf_free_dim_pad_per_rank=0,
        sbuf_byte_offset=0,
    ))


def _build_nc():
    import concourse.bass as bass
    import concourse.bacc as bacc
    import concourse.mybir as mybir
    import concourse.tile as tile
    from concourse.library_config import mlp
    from concourse.masks import make_identity

    f32, f16, bf16, i16 = (mybir.dt.float32, mybir.dt.float16,
                           mybir.dt.bfloat16, mybir.dt.int16)
    AF = mybir.ActivationFunctionType
    OP = mybir.AluOpType

    nc = bacc.Bacc("TRN2", target_bir_lowering=False, debug=False,
                   num_devices=8, num_swdge_queues=4)

    xT = nc.dram_tensor("xT", [2, 128, 100352], f16, kind="ExternalInput")
    xTloc = nc.dram_tensor("xTloc", [2, 128, NLOC], f16, kind="ExternalInput")
    W1e = nc.dram_tensor("W1e", [2, 128, 80], f16, kind="ExternalInput")
    W2e = nc.dram_tensor("W2e", [64, 41], f16, kind="ExternalInput")
    vd2 = nc.dram_tensor("vd2", [1, 64], f16, kind="ExternalInput")
    b1 = nc.dram_tensor("b1", [1, 64], f32, kind="ExternalInput")
    b2 = nc.dram_tensor("b2", [1, 40], f32, kind="ExternalInput")
    hidx = nc.dram_tensor("hidx", [128, NSLOT // 16], i16, kind="ExternalInput")
    drel = nc.dram_tensor("drel", [128, NSLOT // 128], bf16, kind="ExternalInput")
    out = nc.dram_tensor("out", [NLOC, 40], f32, kind="ExternalOutput")

    tab1 = nc.dram_tensor("tab1", [NTOT, 128], f16)          # [h1|as1|pad]
    taba1 = nc.dram_tensor("taba1", [NLOC, 8], bf16)         # ad1 per local dst
    tab2 = nc.dram_tensor("tab2", [NTOT, 64], f32)           # [h2|as2|pad]
    taba2 = nc.dram_tensor("taba2", [NLOC, 8], bf16)         # ad2 in all cols
    agi = nc.dram_tensor("agi", [64, NLOC], f16)
    ago = nc.dram_tensor("ago", [512, NLOC], f16, addr_space="Shared")

    def BC(ap, dims):
        return bass.AP(ap.tensor, ap.offset, dims)

    with tile.TileContext(nc) as tc:
        with tc.tile_pool(name="const", bufs=1) as pc:
            nc.gpsimd.load_library(mlp)

            hidx_sb = pc.tile([128, NSLOT // 16], i16)
            drel_sb = pc.tile([128, NSLOT // 128], bf16)
            nc.sync.dma_start(hidx_sb[:], hidx[:])
            nc.sync.dma_start(drel_sb[:], drel[:])
            w1_sb = pc.tile([128, 2, 80], f16)
            nc.sync.dma_start(w1_sb[:], W1e[:].rearrange("k p n -> p k n"))
            w2_sb = pc.tile([64, 41], f16)
            nc.sync.dma_start(w2_sb[:], W2e[:])

            iota_i = pc.tile([128, 128], i16)
            nc.gpsimd.iota(iota_i[:], pattern=[[1, 128]], base=0,
                           channel_multiplier=0)
            iota_sb = pc.tile([128, 128], bf16)
            nc.vector.tensor_copy(out=iota_sb[:], in_=iota_i[:])

            ident = pc.tile([128, 128], f16)
            make_identity(nc, ident[:])
            identb = pc.tile([128, 128], bf16)
            make_identity(nc, identb[:])

            ones32 = pc.tile([1, 128], f32)
            nc.vector.memset(ones32[:], 1.0)
            ones16 = pc.tile([1, 128], f16)
            nc.vector.memset(ones16[:], 1.0)

            b1r = pc.tile([128, 64], f32)
            b2r = pc.tile([128, 40], f32)
            vd2r = pc.tile([128, 64], f32)
            with tc.tile_pool(name="pini", bufs=2, space="PSUM") as ppi:
                for row_d, width, rdt, dest in (
                        (b1, 64, f32, b1r), (b2, 40, f32, b2r),
                        (vd2, 64, f16, vd2r)):
                    t = pc.tile([1, width], rdt, tag=f"rrow{width}{rdt}")
                    nc.sync.dma_start(t[:], row_d[:])
                    ps = ppi.tile([128, width], f32, tag="rep")
                    lhs = ones32 if rdt == f32 else ones16
                    nc.tensor.matmul(ps[:], lhsT=lhs[:], rhs=t[:],
                                     start=True, stop=True)
                    nc.vector.tensor_copy(out=dest[:], in_=ps[:])

            # ---------- phase A: global node pass -> tab1 ----------
            with (tc.tile_pool(name="pa", bufs=3) as pa,
                  tc.tile_pool(name="ppa", bufs=2, space="PSUM") as ppa):
                for i0 in range(0, NCHK, 4):
                    nb = min(4, NCHK - i0)
                    xt = pa.tile([128, 2, 512], f16, tag="xt")
                    for k in range(2):
                        nc.sync.dma_start(xt[:, k, :nb * 128],
                                          xT[k, :, i0 * 128:(i0 + nb) * 128])
                    row = pa.tile([128, 4, 128], f16, tag="row")
                    for j in range(nb):
                        ps = ppa.tile([128, 80], f32, tag="np1")
                        for k in range(2):
                            nc.tensor.matmul(
                                ps[:], lhsT=xt[:, k, j * 128:(j + 1) * 128],
                                rhs=w1_sb[:, k, :], start=(k == 0),
                                stop=(k == 1))
                        nc.vector.tensor_copy(out=row[:, j, :72],
                                              in_=ps[:, :72])
                        nc.vector.memset(row[:, j, 72:], 0.0)
                    nc.sync.dma_start(
                        tab1[i0 * 128:(i0 + nb) * 128, :].rearrange(
                            "(a b) c -> b a c", b=128),
                        row[:, :nb, :])

                # ---------- phase A2: local pass -> taba1 (ad1) ----------
                for i0 in range(0, NW, 4):
                    xt = pa.tile([128, 2, 512], f16, tag="xt")
                    for k in range(2):
                        nc.sync.dma_start(
                            xt[:, k, :], xTloc[k, :, i0 * 128:(i0 + 4) * 128])
                    ad = pa.tile([128, 4, 8], bf16, tag="ad")
                    for j in range(4):
                        ps = ppa.tile([128, 80], f32, tag="np1")
                        for k in range(2):
                            nc.tensor.matmul(
                                ps[:], lhsT=xt[:, k, j * 128:(j + 1) * 128],
                                rhs=w1_sb[:, k, :], start=(k == 0),
                                stop=(k == 1))
                        nc.vector.tensor_copy(out=ad[:, j, :], in_=ps[:, 72:80])
                    nc.sync.dma_start(
                        taba1[i0 * 128:(i0 + 4) * 128, :].rearrange(
                            "(a b) c -> b a c", b=128),
                        ad[:])

            # ---------- phase B: layer-1 edge pass ----------
            with (tc.tile_pool(name="pb", bufs=3) as pb,
                  tc.tile_pool(name="ppb", bufs=2, space="PSUM") as ppb):
                for w in range(NW):
                    ht = pb.tile([128, CPW, 128], f16, tag="ht")
                    for qi in range(NQ):
                        g = w * NQ + qi
                        nc.gpsimd.dma_gather(
                            ht[:, qi * CPG:(qi + 1) * CPG, :],
                            tab1[qi * QS:(qi + 1) * QS, :],
                            hidx_sb[:, g * (SQ // 16):(g + 1) * (SQ // 16)],
                            SQ, SQ, 128, queue_num=qi)
                    aw1 = pb.tile([128, 8], bf16, tag="aw1")
                    nc.sync.dma_start(aw1[:], taba1[w * 128:(w + 1) * 128, :])

                    dr = drel_sb[:, w * CPW:(w + 1) * CPW]
                    oh = pb.tile([128, CPW, 128], bf16, tag="oh")
                    nc.vector.tensor_tensor(
                        out=oh[:],
                        in0=BC(dr, [dr.ap[0], dr.ap[1], [0, 128]]),
                        in1=BC(iota_sb[:], [iota_sb[:].ap[0], [0, CPW], [1, 128]]),
                        op=OP.is_equal)

                    # alpha_dst expansion: at[:, k, :] = oh[:, k, :] @ aw1
                    at = pb.tile([128, CPW, 8], f32, tag="at")
                    for k in range(CPW):
                        ohp = ppb.tile([128, 128], bf16, tag="ohT")
                        nc.tensor.transpose(out=ohp[:], in_=oh[:, k, :],
                                            identity=identb[:])
                        ohs = pb.tile([128, 128], bf16, tag="ohs")
                        nc.scalar.copy(out=ohs[:], in_=ohp[:])
                        adp = ppb.tile([128, 8], f32, tag="adp")
                        nc.tensor.matmul(adp[:], lhsT=ohs[:], rhs=aw1[:],
                                         start=True, stop=True)
                        nc.vector.tensor_copy(out=at[:, k, :], in_=adp[:])

                    e1 = pb.tile([128, CPW, 8], f32, tag="e1")
                    nc.vector.tensor_tensor(out=e1[:], in0=ht[:, :, 64:72],
                                            in1=at[:], op=OP.add)
                    ls = pb.tile([128, CPW, 8], f32, tag="ls")
                    nc.vector.tensor_scalar_mul(ls[:], e1[:], 0.2)
                    lr = pb.tile([128, CPW, 8], f32, tag="lr")
                    nc.vector.tensor_tensor(out=lr[:], in0=e1[:], in1=ls[:],
                                            op=OP.max)
                    wg = pb.tile([128, CPW, 8], f32, tag="wg")
                    nc.scalar.activation(out=wg[:], in_=lr[:], func=AF.Exp)

                    msg = pb.tile([128, CPW, 72], bf16, tag="msg")
                    mfull = msg[:]
                    hfull = ht[:]
                    wfull = wg[:]
                    nc.vector.tensor_tensor(
                        out=BC(mfull, [mfull.ap[0], [72, CPW], [8, 8], [1, 8]]),
                        in0=BC(hfull, [hfull.ap[0], [128, CPW], [8, 8], [1, 8]]),
                        in1=BC(wfull, [wfull.ap[0], [8, CPW], [1, 8], [0, 8]]),
                        op=OP.mult)
                    nc.vector.tensor_copy(out=msg[:, :, 64:72], in_=wg[:])

                    ps = ppb.tile([128, 72], f32, tag="agg")
                    for k in range(CPW):
                        nc.tensor.matmul(ps[:], lhsT=oh[:, k, :],
                                         rhs=msg[:, k, :], start=(k == 0),
                                         stop=(k == CPW - 1))

                    den = pb.tile([128, 8], f32, tag="den")
                    nc.vector.tensor_scalar_add(den[:], ps[:, 64:72], ACC_EPS)
                    rec = pb.tile([128, 8], f32, tag="rec")
                    nc.vector.reciprocal(rec[:], den[:])
                    o1 = pb.tile([128, 64], f32, tag="o1")
                    pnum = ps[:, 0:64]
                    rfull = rec[:]
                    nc.vector.tensor_tensor(
                        out=BC(o1[:], [o1[:].ap[0], [8, 8], [1, 8]]),
                        in0=BC(pnum, [pnum.ap[0], [8, 8], [1, 8]]),
                        in1=BC(rfull, [rfull.ap[0], [1, 8], [0, 8]]),
                        op=OP.mult)
                    o1b = pb.tile([128, 64], f32, tag="o1b")
                    nc.vector.tensor_tensor(out=o1b[:], in0=o1[:], in1=b1r[:],
                                            op=OP.add)
                    # elu = relu(x) + exp(min(x,0)) - 1
                    t1 = pb.tile([128, 64], f32, tag="t1")
                    nc.vector.tensor_scalar_min(t1[:], o1b[:], 0.0)
                    t2 = pb.tile([128, 64], f32, tag="t2")
                    nc.scalar.activation(out=t2[:], in_=t1[:], func=AF.Exp)
                    t3 = pb.tile([128, 64], f32, tag="t3")
                    nc.vector.tensor_scalar_max(t3[:], o1b[:], 0.0)
                    t4 = pb.tile([128, 64], f32, tag="t4")
                    nc.vector.tensor_tensor(out=t4[:], in0=t2[:], in1=t3[:],
                                            op=OP.add)
                    hl = pb.tile([128, 64], f32, tag="hl")
                    nc.vector.tensor_scalar_add(hl[:], t4[:], -1.0)

                    # ad2 for layer 2
                    t5 = pb.tile([128, 64], f32, tag="t5")
                    nc.vector.tensor_tensor(out=t5[:], in0=hl[:], in1=vd2r[:],
                                            op=OP.mult)
                    ad2 = pb.tile([128, 1], f32, tag="ad2")
                    nc.vector.tensor_reduce(ad2[:], t5[:],
                                            axis=mybir.AxisListType.X,
                                            op=OP.add)
                    ad2b = pb.tile([128, 8], bf16, tag="ad2b")
                    nc.vector.tensor_copy(out=ad2b[:],
                                          in_=ad2[:].to_broadcast([128, 8]))
                    nc.sync.dma_start(taba2[w * 128:(w + 1) * 128, :], ad2b[:])

                    hl16 = pb.tile([128, 64], f16, tag="hl16")
                    nc.vector.tensor_copy(out=hl16[:], in_=hl[:])
                    pst = ppb.tile([64, 128], f16, tag="tr")
                    nc.tensor.transpose(out=pst[:], in_=hl16[:], identity=ident[:])
                    hlT = pb.tile([64, 128], f16, tag="hlT")
                    nc.vector.tensor_copy(out=hlT[:], in_=pst[:])
                    nc.sync.dma_start(agi[:, w * 128:(w + 1) * 128], hlT[:])

            # ---------- AllGather ----------
            nc.gpsimd.collective_compute(
                "AllGather", OP.bypass, ins=[agi[:]], outs=[ago[:]],
                replica_groups=[list(range(8))])

            # ---------- phase C: layer-2 node pass -> tab2 ----------
            with (tc.tile_pool(name="pcn", bufs=3) as pn,
                  tc.tile_pool(name="ppc", bufs=2, space="PSUM") as ppc):
                for i0 in range(0, NTOT // 128, 4):
                    cn = (i0 * 128) // NLOC
                    lo = (i0 * 128) % NLOC
                    hT = pn.tile([64, 512], f16, tag="hT")
                    nc.sync.dma_start(hT[:],
                                      ago[cn * 64:(cn + 1) * 64, lo:lo + 512])
                    r2 = pn.tile([128, 4, 41], f32, tag="r2")
                    for j in range(4):
                        ps2 = ppc.tile([128, 41], f32, tag="np2")
                        nc.tensor.matmul(ps2[:],
                                         lhsT=hT[:, j * 128:(j + 1) * 128],
                                         rhs=w2_sb[:], start=True, stop=True)
                        nc.vector.tensor_copy(out=r2[:, j, :], in_=ps2[:])
                    nc.sync.dma_start(
                        tab2[i0 * 128:(i0 + 4) * 128, 0:41].rearrange(
                            "(a b) c -> b a c", b=128),
                        r2[:])

            # ---------- phase D: layer-2 edge pass -> out ----------
            with (tc.tile_pool(name="pd", bufs=3) as pd,
                  tc.tile_pool(name="ppd", bufs=2, space="PSUM") as ppd):
                for w in range(NW):
                    g2 = pd.tile([128, CPW, 64], f32, tag="g2")
                    for qi in range(NQ):
                        g = w * NQ + qi
                        nc.gpsimd.dma_gather(
                            g2[:, qi * CPG:(qi + 1) * CPG, :],
                            tab2[qi * QS:(qi + 1) * QS, :],
                            hidx_sb[:, g * (SQ // 16):(g + 1) * (SQ // 16)],
                            SQ, SQ, 64, queue_num=qi)
                    aw2 = pd.tile([128, 8], bf16, tag="aw2")
                    nc.sync.dma_start(aw2[:], taba2[w * 128:(w + 1) * 128, :])

                    dr = drel_sb[:, w * CPW:(w + 1) * CPW]
                    oh = pd.tile([128, CPW, 128], bf16, tag="oh2")
                    nc.vector.tensor_tensor(
                        out=oh[:],
                        in0=BC(dr, [dr.ap[0], dr.ap[1], [0, 128]]),
                        in1=BC(iota_sb[:], [iota_sb[:].ap[0], [0, CPW], [1, 128]]),
                        op=OP.is_equal)

                    a2 = pd.tile([128, CPW, 1], f32, tag="a2")
                    for k in range(CPW):
                        ohp = ppd.tile([128, 128], bf16, tag="ohT2")
                        nc.tensor.transpose(out=ohp[:], in_=oh[:, k, :],
                                            identity=identb[:])
                        ohs = pd.tile([128, 128], bf16, tag="ohs2")
                        nc.scalar.copy(out=ohs[:], in_=ohp[:])
                        adp = ppd.tile([128, 1], f32, tag="adp2")
                        nc.tensor.matmul(adp[:], lhsT=ohs[:], rhs=aw2[:, 0:1],
                                         start=True, stop=True)
                        nc.vector.tensor_copy(out=a2[:, k, :], in_=adp[:])

                    e2 = pd.tile([128, CPW, 1], f32, tag="e2")
                    nc.vector.tensor_tensor(out=e2[:], in0=g2[:, :, 40:41],
                                            in1=a2[:], op=OP.add)
                    ls2 = pd.tile([128, CPW, 1], f32, tag="ls2")
                    nc.vector.tensor_scalar_mul(ls2[:], e2[:], 0.2)
                    lr2 = pd.tile([128, CPW, 1], f32, tag="lr2")
                    nc.vector.tensor_tensor(out=lr2[:], in0=e2[:], in1=ls2[:],
                                            op=OP.max)
                    wg2 = pd.tile([128, CPW, 1], f32, tag="wg2")
                    nc.scalar.activation(out=wg2[:], in_=lr2[:], func=AF.Exp)

                    m2 = pd.tile([128, CPW, 41], bf16, tag="m2")
                    m2f = m2[:]
                    g2f = g2[:]
                    w2f = wg2[:]
                    nc.vector.tensor_tensor(
                        out=BC(m2f, [m2f.ap[0], [41, CPW], [1, 40]]),
                        in0=BC(g2f, [g2f.ap[0], [64, CPW], [1, 40]]),
                        in1=BC(w2f, [w2f.ap[0], [1, CPW], [0, 40]]),
                        op=OP.mult)
                    nc.vector.tensor_copy(out=m2[:, :, 40:41], in_=wg2[:])

                    ps = ppd.tile([128, 41], f32, tag="agg2")
                    for k in range(CPW):
                        nc.tensor.matmul(ps[:], lhsT=oh[:, k, :],
                                         rhs=m2[:, k, :], start=(k == 0),
                                         stop=(k == CPW - 1))

                    den2 = pd.tile([128, 1], f32, tag="den2")
                    nc.vector.tensor_scalar_add(den2[:], ps[:, 40:41], ACC_EPS)
                    rec2 = pd.tile([128, 1], f32, tag="rec2")
                    nc.vector.reciprocal(rec2[:], den2[:])
                    o2 = pd.tile([128, 40], f32, tag="o2")
                    nc.vector.tensor_tensor(out=o2[:], in0=ps[:, 0:40],
                                            in1=rec2[:].to_broadcast([128, 40]),
                                            op=OP.mult)
                    o2b = pd.tile([128, 40], f32, tag="o2b")
                    nc.vector.tensor_tensor(out=o2b[:], in0=o2[:], in1=b2r[:],
                                            op=OP.add)
                    nc.sync.dma_start(out[w * 128:(w + 1) * 128, :], o2b[:])

    nc.finalize()
    return nc


def kernel(**inputs):
    per_core = _host_prep(**inputs)
    if "nc" not in _CACHE:
        _CACHE["nc"] = _build_nc()
    nc = _CACHE["nc"]
    from concourse.bass_utils import run_bass_kernel_spmd
    res = run_bass_kernel_spmd(nc, per_core, list(range(8)))
    full = np.concatenate([res.results[c]["out"] for c in range(8)], axis=0)
    return np.ascontiguousarray(full[:N]).astype(np.float32)



# revision 2
# speedup vs baseline: 1.9981x; 1.9981x over previous
"""GAT (2-layer PyG GATConv, eval) on 8 Trainium2 NeuronCores — v2.

Sharding: nodes range-partitioned (NLOC=12800/core); each core owns edges whose
dst is in its range. Edges grouped block-major: (block of B=10 windows,
src-quadrant, window, 640-slot group). Per block: 4 batched dma_gathers fetch
[h|1.0|as] rows (160B payload) from the layer table + 1 small gather fetches
per-slot alpha_dst (16B) from a local per-dst table — no per-chunk transpose
chains. Layer tables are built by a sliced node pass (1/8 nodes per core) and
AllGathered. Softmax uses the shift-invariant no-max form; the denominator
comes free from a 1.0 column folded into the gathered rows. Segment reduction
is a PE matmul with one-hot(dst_rel) per 128-slot chunk.
"""
import numpy as np
import ml_dtypes

N = 100000
E = 1600000
NF = 256
HEADS, NHID = 8, 8
NCLASS = 40
NLOC = 12800               # nodes per core
NW = 100                   # 128-dst windows per core
B = 10                     # windows per block
NBLK = NW // B             # 10 blocks per core
NQ = 4                     # src quadrants
QS = 25600                 # rows per quadrant sub-table
SQ = 640                   # edge slots per (window, quadrant) group
NSLOT = NBLK * NQ * B * SQ  # 256000 slots per core
NTOT = 102400
ACC_EPS = 1e-16

_CACHE = {}


def _host_prep(x, edge_index, W1, a1_src, a1_dst, b1, W2, a2_src, a2_dst, b2):
    src = np.asarray(edge_index[0], dtype=np.int64)
    dst = np.asarray(edge_index[1], dtype=np.int64)

    core = dst // NLOC
    dloc = dst - core * NLOC
    win = dloc >> 7
    q = src // QS
    blk = win // B
    wi = win - blk * B
    gid = ((core * NBLK + blk) * NQ + q) * B + wi
    order = np.argsort(gid, kind="stable")
    gsz = np.bincount(gid, minlength=8 * NBLK * NQ * B)
    assert gsz.max() <= SQ, f"group overflow: {gsz.max()} > {SQ}"
    starts = np.zeros_like(gsz)
    starts[1:] = np.cumsum(gsz)[:-1]
    g_sorted = gid[order]
    rank = np.arange(E) - starts[g_sorted]
    slot_global = g_sorted * SQ + rank
    cc = slot_global // NSLOT
    sc = slot_global - cc * NSLOT

    hidx = np.zeros((8, NSLOT), np.int16)          # pad -> row 0 of sub-table
    adix = np.zeros((8, NSLOT), np.int16)          # pad -> local row 0
    drel = np.full((8, NSLOT), 128.0, np.float32)  # pad -> out-of-window
    hidx[cc, sc] = (src[order] - q[order] * QS).astype(np.int16)
    adix[cc, sc] = dloc[order].astype(np.int16)
    drel[cc, sc] = (dloc[order] & 127).astype(np.float32)

    # dma_gather idx wrap: token s -> [s%16, s//16], replicated into all eight
    # 16-partition blocks (Q7 cpu pairs for the 4 SWDGE queues).
    slots = np.arange(NSLOT)

    def wrap16(a):
        w = np.zeros((8, 128, NSLOT // 16), np.int16)
        w[:, slots % 16, slots // 16] = a
        for r in range(1, 8):
            w[:, 16 * r:16 * (r + 1)] = w[:, :16]
        return w

    hw = wrap16(hidx)
    aw = wrap16(adix)

    # dst_rel, chunk-major: token s -> [s%128, s//128]
    dw = np.zeros((8, 128, NSLOT // 128), np.float32)
    dw[:, slots % 128, slots // 128] = drel
    dw = dw.astype(ml_dtypes.bfloat16)

    # fold attention vectors into the node-pass weights
    W1 = np.asarray(W1, np.float32)
    v_s1 = np.einsum("chk,hk->ch", W1.reshape(NF, HEADS, NHID),
                     np.asarray(a1_src, np.float32))
    v_d1 = np.einsum("chk,hk->ch", W1.reshape(NF, HEADS, NHID),
                     np.asarray(a1_dst, np.float32))
    W1e = np.concatenate([W1, v_s1, v_d1], axis=1).reshape(2, 128, 80)
    W1e = W1e.astype(np.float16)

    W2 = np.asarray(W2, np.float32)
    v_s2 = W2 @ np.asarray(a2_src, np.float32)[0]
    v_d2 = W2 @ np.asarray(a2_dst, np.float32)[0]
    W2e = np.concatenate([W2, v_s2[:, None], v_d2[:, None]],
                         axis=1).astype(np.float16)       # [64, 42]

    xp = np.zeros((NTOT, NF), np.float32)
    xp[:N] = np.asarray(x, np.float32)

    per_core = []
    for c in range(8):
        xloc = np.ascontiguousarray(xp[c * NLOC:(c + 1) * NLOC].T)
        per_core.append({
            "xTloc": xloc.astype(np.float16).reshape(2, 128, NLOC),
            "W1e": W1e,
            "W2e": W2e,
            "b1": np.asarray(b1, np.float32)[None, :],
            "b2": np.asarray(b2, np.float32)[None, :],
            "hidx": hw[c],
            "adix": aw[c],
            "drel": dw[c],
        })
    return per_core


def _dma_gather_small(g, out_ap, in_ap, idxs_ap, num_idxs, elem_size,
                      elem_step, queue_num=0):
    """dma_gather with an arbitrary (<256B) payload. Only the row STRIDE must
    be a 256B multiple on the Q7 side; bass's elem_size%256 assert is
    over-conservative for the non-transpose path, so build the instruction
    directly."""
    import concourse.mybir as mybir
    stride_bytes = elem_step * mybir.dt.size(in_ap.dtype)
    assert stride_bytes % 256 == 0
    _in_ap = g.lower_ap_dma(in_ap, for_custom_bir_dma=True)
    _idxs_ap = g.lower_ap(idxs_ap)
    _out_ap = g.lower_ap(out_ap)
    return g.add_instruction(mybir.InstDMAGatherAnt(
        name=g.bass.get_next_instruction_name(),
        ins=[*_in_ap, _idxs_ap, g.lower_val_access(g.to_reg(num_idxs))],
        outs=[_out_ap],
        transpose=False,
        num_idxs=num_idxs,
        elem_size=elem_size,
        stride_bytes_256=stride_bytes // 256,
        gen_mode=0,
        single_packet=True,
        queue_num=queue_num,
        sbuf_tokens_per_rank=0,
        sbuf_free_dim_per_rank=0,
        sbuf_free_dim_pad_per_rank=0,
        sbuf_byte_offset=0,
    ))


def _build_nc():
    import concourse.bass as bass
    import concourse.bacc as bacc
    import concourse.mybir as mybir
    import concourse.tile as tile
    from concourse.library_config import mlp
    from concourse.masks import make_identity

    f32, f16, bf16, i16 = (mybir.dt.float32, mybir.dt.float16,
                           mybir.dt.bfloat16, mybir.dt.int16)
    AF = mybir.ActivationFunctionType
    OP = mybir.AluOpType

    nc = bacc.Bacc("TRN2", target_bir_lowering=False, debug=False,
                   num_devices=8, num_swdge_queues=4)

    xTloc = nc.dram_tensor("xTloc", [2, 128, NLOC], f16, kind="ExternalInput")
    W1e = nc.dram_tensor("W1e", [2, 128, 80], f16, kind="ExternalInput")
    W2e = nc.dram_tensor("W2e", [64, 42], f16, kind="ExternalInput")
    b1 = nc.dram_tensor("b1", [1, 64], f32, kind="ExternalInput")
    b2 = nc.dram_tensor("b2", [1, 40], f32, kind="ExternalInput")
    hidx = nc.dram_tensor("hidx", [128, NSLOT // 16], i16, kind="ExternalInput")
    adix = nc.dram_tensor("adix", [128, NSLOT // 16], i16, kind="ExternalInput")
    drel = nc.dram_tensor("drel", [128, NSLOT // 128], bf16, kind="ExternalInput")
    out = nc.dram_tensor("out", [NLOC, 40], f32, kind="ExternalOutput")

    tabin = nc.dram_tensor("tabin", [NLOC, 128], f16)    # local [h|1|as] rows
    taba = nc.dram_tensor("taba", [NLOC, 128], f16)      # ad1 cols 0:8, ad2 col 8
    tab2in = nc.dram_tensor("tab2in", [NLOC, 128], f16)  # local [h2|1|as2] rows
    tab1 = nc.dram_tensor("tab1", [NTOT, 128], f16, addr_space="Shared")
    tab2 = nc.dram_tensor("tab2", [NTOT, 128], f16, addr_space="Shared")

    def BC(ap, dims):
        return bass.AP(ap.tensor, ap.offset, dims)

    with tile.TileContext(nc) as tc:
        with tc.tile_pool(name="const", bufs=1) as pc:
            nc.gpsimd.load_library(mlp)

            adix_sb = pc.tile([128, NSLOT // 16], i16)
            drel_sb = pc.tile([128, NSLOT // 128], bf16)
            nc.sync.dma_start(adix_sb[:], adix[:])
            nc.sync.dma_start(drel_sb[:], drel[:])
            w1_sb = pc.tile([128, 2, 80], f16)
            nc.sync.dma_start(w1_sb[:], W1e[:].rearrange("k p n -> p k n"))
            w2_sb = pc.tile([64, 42], f16)
            nc.sync.dma_start(w2_sb[:], W2e[:])

            iota_i = pc.tile([128, 128], i16)
            nc.gpsimd.iota(iota_i[:], pattern=[[1, 128]], base=0,
                           channel_multiplier=0)
            iota_sb = pc.tile([128, 128], bf16)
            nc.vector.tensor_copy(out=iota_sb[:], in_=iota_i[:])
            # iotaT5[p, d*5+j] = d
            iotaT5 = pc.tile([128, 128, 5], bf16)
            nc.vector.tensor_copy(
                out=iotaT5[:],
                in_=BC(iota_sb[:], [iota_sb[:].ap[0], [1, 128], [0, 5]]))

            ident = pc.tile([128, 128], f16)
            make_identity(nc, ident[:])

            ones32 = pc.tile([1, 128], f32)
            nc.vector.memset(ones32[:], 1.0)

            b1r = pc.tile([128, 64], f32)
            b2r = pc.tile([128, 40], f32)
            with tc.tile_pool(name="pini", bufs=2, space="PSUM") as ppi:
                for row_d, width, dest in ((b1, 64, b1r), (b2, 40, b2r)):
                    t = pc.tile([1, width], f32, tag=f"rrow{width}")
                    nc.sync.dma_start(t[:], row_d[:])
                    ps = ppi.tile([128, width], f32, tag="rep")
                    nc.tensor.matmul(ps[:], lhsT=ones32[:], rhs=t[:],
                                     start=True, stop=True)
                    nc.vector.tensor_copy(out=dest[:], in_=ps[:])

            # ---------- phase A: sliced node pass -> tabin, taba ----------
            with (tc.tile_pool(name="pa", bufs=3) as pa,
                  tc.tile_pool(name="ppa", bufs=2, space="PSUM") as ppa):
                for i0 in range(0, NW, 4):
                    xt = pa.tile([128, 2, 512], f16, tag="xt")
                    for k in range(2):
                        nc.sync.dma_start(
                            xt[:, k, :], xTloc[k, :, i0 * 128:(i0 + 4) * 128])
                    slab = pa.tile([128, 4, 80], f16, tag="slab")
                    nc.vector.memset(slab[:], 1.0)
                    ad4 = pa.tile([128, 4, 8], f16, tag="ad4")
                    for j in range(4):
                        ps = ppa.tile([128, 80], f32, tag="np1")
                        for k in range(2):
                            nc.tensor.matmul(
                                ps[:], lhsT=xt[:, k, j * 128:(j + 1) * 128],
                                rhs=w1_sb[:, k, :], start=(k == 0),
                                stop=(k == 1))
                        sj = slab[:, j, :]
                        nc.vector.tensor_copy(
                            out=BC(sj, [sj.ap[0], [9, 8], [1, 8]]),
                            in_=ps[:, 0:64])
                        nc.vector.tensor_copy(out=slab[:, j, 72:80],
                                              in_=ps[:, 64:72])
                        nc.vector.tensor_copy(out=ad4[:, j, :],
                                              in_=ps[:, 72:80])
                    nc.sync.dma_start(
                        tabin[i0 * 128:(i0 + 4) * 128, 0:80].rearrange(
                            "(a b) c -> b a c", b=128),
                        slab[:])
                    nc.sync.dma_start(
                        taba[i0 * 128:(i0 + 4) * 128, 0:8].rearrange(
                            "(a b) c -> b a c", b=128),
                        ad4[:])

            # ---------- AllGather layer-1 table ----------
            nc.gpsimd.collective_compute(
                "AllGather", OP.bypass, ins=[tabin[:]], outs=[tab1[:]],
                replica_groups=[list(range(8))])

            # ---------- phase B: layer-1 edge pass ----------
            with (tc.tile_pool(name="pat", bufs=1) as pat,
                  tc.tile_pool(name="pb", bufs=2) as pb,
                  tc.tile_pool(name="pw", bufs=3) as pw,
                  tc.tile_pool(name="ppb", bufs=2, space="PSUM") as ppb):
                # per-slot alpha_dst for all blocks (overlaps the AllGather)
                at_t = []
                for b in range(NBLK):
                    at = pat.tile([128, NQ * B * 5, 8], f16, tag=f"at{b}")
                    _dma_gather_small(
                        nc.gpsimd, at[:], taba[:, 0:8],
                        adix_sb[:, b * 1600:(b + 1) * 1600],
                        NQ * B * SQ, 8, 128, queue_num=b % 4)
                    at_t.append(at)

                for b in range(NBLK):
                    hix = pb.tile([128, 1600], i16, tag="hix")
                    nc.sync.dma_start(hix[:],
                                      hidx[:, b * 1600:(b + 1) * 1600])
                    ht = pb.tile([128, NQ, B * 5, 80], f16, tag="ht")
                    for qi in range(NQ):
                        _dma_gather_small(
                            nc.gpsimd, ht[:, qi], tab1[qi * QS:(qi + 1) * QS, 0:80],
                            hix[:, qi * 400:(qi + 1) * 400],
                            B * SQ, 80, 128, queue_num=qi)
                    at = at_t[b]

                    for wi in range(B):
                        w = b * B + wi
                        # one-hot (slots x dst_rel), [128, 128, NQ, 5]
                        oh = pw.tile([128, 128, NQ, 5], bf16, tag="oh")
                        dr = drel_sb[:, b * 200 + wi * 5:(b + 1) * 200]
                        nc.vector.tensor_tensor(
                            out=oh[:],
                            in0=BC(dr, [dr.ap[0], [0, 128], [50, NQ], [1, 5]]),
                            in1=BC(iotaT5[:],
                                   [iotaT5[:].ap[0], [5, 128], [0, NQ], [1, 5]]),
                            op=OP.is_equal)

                        e1 = pw.tile([128, NQ, 5, 8], f16, tag="e1")
                        hslice = ht[:, :, wi * 5:(wi + 1) * 5, 72:80]
                        aslice = at[:, wi * 5:, :]
                        nc.vector.tensor_tensor(
                            out=e1[:], in0=hslice,
                            in1=BC(aslice, [aslice.ap[0], [400, NQ], [8, 5], [1, 8]]),
                            op=OP.add)
                        lr = pw.tile([128, NQ, 5, 8], f16, tag="lr")
                        nc.scalar.activation(out=lr[:], in_=e1[:],
                                             func=AF.Lrelu, alpha=0.2)
                        wg = pw.tile([128, NQ, 5, 8], f32, tag="wg")
                        nc.scalar.activation(out=wg[:], in_=lr[:], func=AF.Exp)

                        # msg[s, q, j, h*9+k] = ht * wg (k=8 col gives w itself)
                        msg = pw.tile([128, NQ, 5, 72], bf16, tag="msg")
                        for qi in range(NQ):
                            mq = msg[:, qi]
                            hq = ht[:, qi, wi * 5:(wi + 1) * 5, 0:72]
                            wq = wg[:, qi]
                            nc.vector.tensor_tensor(
                                out=BC(mq, [mq.ap[0], [72, 5], [9, 8], [1, 9]]),
                                in0=BC(hq, [hq.ap[0], [80, 5], [9, 8], [1, 9]]),
                                in1=BC(wq, [wq.ap[0], [8, 5], [1, 8], [0, 9]]),
                                op=OP.mult)

                        ps = ppb.tile([128, 72], f32, tag="agg")
                        for k in range(NQ * 5):
                            qi, j = divmod(k, 5)
                            nc.tensor.matmul(ps[:], lhsT=oh[:, :, qi, j],
                                             rhs=msg[:, qi, j, :],
                                             start=(k == 0),
                                             stop=(k == NQ * 5 - 1))

                        den = pw.tile([128, 8], f32, tag="den")
                        pden = ps[:, 8:72]
                        nc.vector.tensor_scalar_add(
                            den[:], BC(pden, [pden.ap[0], [9, 8]]), ACC_EPS)
                        rec = pw.tile([128, 8], f32, tag="rec")
                        nc.vector.reciprocal(rec[:], den[:])
                        o1 = pw.tile([128, 64], f32, tag="o1")
                        pnum = ps[:, 0:72]
                        rfull = rec[:]
                        nc.vector.tensor_tensor(
                            out=BC(o1[:], [o1[:].ap[0], [8, 8], [1, 8]]),
                            in0=BC(pnum, [pnum.ap[0], [9, 8], [1, 8]]),
                            in1=BC(rfull, [rfull.ap[0], [1, 8], [0, 8]]),
                            op=OP.mult)
                        o1b = pw.tile([128, 64], f32, tag="o1b")
                        nc.vector.tensor_tensor(out=o1b[:], in0=o1[:],
                                                in1=b1r[:], op=OP.add)
                        # elu = relu(x) + exp(-relu(-x)) - 1
                        u = pw.tile([128, 64], f32, tag="u")
                        nc.scalar.activation(out=u[:], in_=o1b[:],
                                             func=AF.Relu, scale=-1.0)
                        t2 = pw.tile([128, 64], f32, tag="t2")
                        nc.scalar.activation(out=t2[:], in_=u[:],
                                             func=AF.Exp, scale=-1.0)
                        t3 = pw.tile([128, 64], f32, tag="t3")
                        nc.scalar.activation(out=t3[:], in_=o1b[:], func=AF.Relu)
                        t4 = pw.tile([128, 64], f32, tag="t4")
                        nc.vector.tensor_tensor(out=t4[:], in0=t2[:], in1=t3[:],
                                                op=OP.add)
                        hl16 = pw.tile([128, 64], f16, tag="hl16")
                        nc.vector.tensor_scalar_add(hl16[:], t4[:], -1.0)

                        pst = ppb.tile([64, 128], f16, tag="tr")
                        nc.tensor.transpose(out=pst[:], in_=hl16[:],
                                            identity=ident[:])
                        hlT = pw.tile([64, 128], f16, tag="hlT")
                        nc.scalar.copy(out=hlT[:], in_=pst[:])
                        ps2 = ppb.tile([128, 42], f32, tag="mm2")
                        nc.tensor.matmul(ps2[:], lhsT=hlT[:], rhs=w2_sb[:],
                                         start=True, stop=True)
                        t2row = pw.tile([128, 42], f16, tag="t2row")
                        nc.vector.memset(t2row[:, 40:41], 1.0)
                        nc.scalar.copy(out=t2row[:, 0:40], in_=ps2[:, 0:40])
                        nc.scalar.copy(out=t2row[:, 41:42], in_=ps2[:, 40:41])
                        ad2t = pw.tile([128, 1], f16, tag="ad2t")
                        nc.scalar.copy(out=ad2t[:], in_=ps2[:, 41:42])
                        nc.sync.dma_start(
                            tab2in[w * 128:(w + 1) * 128, 0:42], t2row[:])
                        nc.sync.dma_start(
                            taba[w * 128:(w + 1) * 128, 8:9], ad2t[:])

            # ---------- AllGather layer-2 table ----------
            nc.gpsimd.collective_compute(
                "AllGather", OP.bypass, ins=[tab2in[:]], outs=[tab2[:]],
                replica_groups=[list(range(8))])

            # ---------- phase D: layer-2 edge pass -> out ----------
            with (tc.tile_pool(name="pat2", bufs=1) as pat2,
                  tc.tile_pool(name="pd", bufs=2) as pd,
                  tc.tile_pool(name="pw2", bufs=3) as pw2,
                  tc.tile_pool(name="ppd", bufs=2, space="PSUM") as ppd):
                at2_t = []
                for b in range(NBLK):
                    at2 = pat2.tile([128, NQ * B * 5, 1], f16, tag=f"at2{b}")
                    _dma_gather_small(
                        nc.gpsimd, at2[:], taba[:, 8:9],
                        adix_sb[:, b * 1600:(b + 1) * 1600],
                        NQ * B * SQ, 1, 128, queue_num=b % 4)
                    at2_t.append(at2)

                for b in range(NBLK):
                    hix = pd.tile([128, 1600], i16, tag="hix2")
                    nc.sync.dma_start(hix[:],
                                      hidx[:, b * 1600:(b + 1) * 1600])
                    g2 = pd.tile([128, NQ, B * 5, 42], f16, tag="g2")
                    for qi in range(NQ):
                        _dma_gather_small(
                            nc.gpsimd, g2[:, qi], tab2[qi * QS:(qi + 1) * QS, 0:42],
                            hix[:, qi * 400:(qi + 1) * 400],
                            B * SQ, 42, 128, queue_num=qi)
                    at2 = at2_t[b]

                    for wi in range(B):
                        w = b * B + wi
                        oh = pw2.tile([128, 128, NQ, 5], bf16, tag="oh2")
                        dr = drel_sb[:, b * 200 + wi * 5:(b + 1) * 200]
                        nc.vector.tensor_tensor(
                            out=oh[:],
                            in0=BC(dr, [dr.ap[0], [0, 128], [50, NQ], [1, 5]]),
                            in1=BC(iotaT5[:],
                                   [iotaT5[:].ap[0], [5, 128], [0, NQ], [1, 5]]),
                            op=OP.is_equal)

                        e2 = pw2.tile([128, NQ, 5], f32, tag="e2")
                        gsl = g2[:, :, wi * 5:(wi + 1) * 5, 41:42]
                        asl = at2[:, wi * 5:, :]
                        nc.vector.tensor_tensor(
                            out=e2[:],
                            in0=BC(gsl, [gsl.ap[0], [2100, NQ], [42, 5]]),
                            in1=BC(asl, [asl.ap[0], [50, NQ], [1, 5]]),
                            op=OP.add)
                        lr2 = pw2.tile([128, NQ, 5], f32, tag="lr2")
                        nc.scalar.activation(out=lr2[:], in_=e2[:],
                                             func=AF.Lrelu, alpha=0.2)
                        wg2 = pw2.tile([128, NQ, 5], f32, tag="wg2")
                        nc.scalar.activation(out=wg2[:], in_=lr2[:], func=AF.Exp)

                        m2 = pw2.tile([128, NQ, 5, 41], bf16, tag="m2")
                        g2w = g2[:, :, wi * 5:(wi + 1) * 5, 0:41]
                        w2f = wg2[:]
                        nc.vector.tensor_tensor(
                            out=m2[:], in0=g2w,
                            in1=BC(w2f, [w2f.ap[0], [5, NQ], [1, 5], [0, 41]]),
                            op=OP.mult)

                        ps = ppd.tile([128, 41], f32, tag="agg2")
                        for k in range(NQ * 5):
                            qi, j = divmod(k, 5)
                            nc.tensor.matmul(ps[:], lhsT=oh[:, :, qi, j],
                                             rhs=m2[:, qi, j, :],
                                             start=(k == 0),
                                             stop=(k == NQ * 5 - 1))

                        den2 = pw2.tile([128, 1], f32, tag="den2")
                        nc.vector.tensor_scalar_add(den2[:], ps[:, 40:41],
                                                    ACC_EPS)
                        rec2 = pw2.tile([128, 1], f32, tag="rec2")
                        nc.vector.reciprocal(rec2[:], den2[:])
                        o2 = pw2.tile([128, 40], f32, tag="o2")
                        nc.vector.tensor_scalar_mul(o2[:], ps[:, 0:40], rec2[:])
                        o2b = pw2.tile([128, 40], f32, tag="o2b")
                        nc.vector.tensor_tensor(out=o2b[:], in0=o2[:],
                                                in1=b2r[:], op=OP.add)
                        nc.sync.dma_start(out[w * 128:(w + 1) * 128, :], o2b[:])

    nc.finalize()
    return nc


def kernel(**inputs):
    per_core = _host_prep(**inputs)
    if "nc" not in _CACHE:
        _CACHE["nc"] = _build_nc()
    nc = _CACHE["nc"]
    from concourse.bass_utils import run_bass_kernel_spmd
    res = run_bass_kernel_spmd(nc, per_core, list(range(8)))
    full = np.concatenate([res.results[c]["out"] for c in range(8)], axis=0)
    return np.ascontiguousarray(full[:N]).astype(np.float32)


# revision 3
# speedup vs baseline: 2.0859x; 1.0439x over previous
"""GAT (2-layer PyG GATConv, eval) on 8 Trainium2 NeuronCores — v2.

Sharding: nodes range-partitioned (NLOC=12800/core); each core owns edges whose
dst is in its range. Edges grouped block-major: (block of B=10 windows,
src-quadrant, window, 640-slot group). Per block: 4 batched dma_gathers fetch
[h|1.0|as] rows (160B payload) from the layer table + 1 small gather fetches
per-slot alpha_dst (16B) from a local per-dst table — no per-chunk transpose
chains. Layer tables are built by a sliced node pass (1/8 nodes per core) and
AllGathered. Softmax uses the shift-invariant no-max form; the denominator
comes free from a 1.0 column folded into the gathered rows. Segment reduction
is a PE matmul with one-hot(dst_rel) per 128-slot chunk.
"""
import numpy as np
import ml_dtypes

N = 100000
E = 1600000
NF = 256
HEADS, NHID = 8, 8
NCLASS = 40
NLOC = 12800               # nodes per core
NW = 100                   # 128-dst windows per core
B = 10                     # windows per block
NBLK = NW // B             # 10 blocks per core
NQ = 4                     # src quadrants
QS = 25600                 # rows per quadrant sub-table
SQ = 640                   # edge slots per (window, quadrant) group
NSLOT = NBLK * NQ * B * SQ  # 256000 slots per core
NTOT = 102400
ACC_EPS = 1e-16

_CACHE = {}


def _host_prep(x, edge_index, W1, a1_src, a1_dst, b1, W2, a2_src, a2_dst, b2):
    src = np.asarray(edge_index[0], dtype=np.int64)
    dst = np.asarray(edge_index[1], dtype=np.int64)

    core = dst // NLOC
    dloc = dst - core * NLOC
    win = dloc >> 7
    q = src // QS
    blk = win // B
    wi = win - blk * B
    gid = ((core * NBLK + blk) * NQ + q) * B + wi
    order = np.argsort(gid, kind="stable")
    gsz = np.bincount(gid, minlength=8 * NBLK * NQ * B)
    assert gsz.max() <= SQ, f"group overflow: {gsz.max()} > {SQ}"
    starts = np.zeros_like(gsz)
    starts[1:] = np.cumsum(gsz)[:-1]
    g_sorted = gid[order]
    rank = np.arange(E) - starts[g_sorted]
    slot_global = g_sorted * SQ + rank
    cc = slot_global // NSLOT
    sc = slot_global - cc * NSLOT

    hidx = np.zeros((8, NSLOT), np.int16)          # pad -> row 0 of sub-table
    adix = np.zeros((8, NSLOT), np.int16)          # pad -> local row 0
    drel = np.full((8, NSLOT), 128.0, np.float32)  # pad -> out-of-window
    hidx[cc, sc] = (src[order] - q[order] * QS).astype(np.int16)
    adix[cc, sc] = dloc[order].astype(np.int16)
    drel[cc, sc] = (dloc[order] & 127).astype(np.float32)

    # dma_gather idx wrap: token s -> [s%16, s//16], replicated into all eight
    # 16-partition blocks (Q7 cpu pairs for the 4 SWDGE queues).
    slots = np.arange(NSLOT)

    def wrap16(a):
        w = np.zeros((8, 128, NSLOT // 16), np.int16)
        w[:, slots % 16, slots // 16] = a
        for r in range(1, 8):
            w[:, 16 * r:16 * (r + 1)] = w[:, :16]
        return w

    hw = wrap16(hidx)
    aw = wrap16(adix)

    # dst_rel, chunk-major: token s -> [s%128, s//128]
    dw = np.zeros((8, 128, NSLOT // 128), np.float32)
    dw[:, slots % 128, slots // 128] = drel
    dw = dw.astype(ml_dtypes.bfloat16)

    # fold attention vectors into the node-pass weights
    W1 = np.asarray(W1, np.float32)
    v_s1 = np.einsum("chk,hk->ch", W1.reshape(NF, HEADS, NHID),
                     np.asarray(a1_src, np.float32))
    v_d1 = np.einsum("chk,hk->ch", W1.reshape(NF, HEADS, NHID),
                     np.asarray(a1_dst, np.float32))
    W1e = np.concatenate([W1, v_s1, v_d1], axis=1).reshape(2, 128, 80)
    W1e = W1e.astype(np.float16)

    W2 = np.asarray(W2, np.float32)
    v_s2 = W2 @ np.asarray(a2_src, np.float32)[0]
    v_d2 = W2 @ np.asarray(a2_dst, np.float32)[0]
    W2e = np.concatenate([W2, v_s2[:, None], v_d2[:, None]],
                         axis=1).astype(np.float16)       # [64, 42]

    xp = np.zeros((NTOT, NF), np.float32)
    xp[:N] = np.asarray(x, np.float32)

    per_core = []
    for c in range(8):
        xloc = np.ascontiguousarray(xp[c * NLOC:(c + 1) * NLOC].T)
        per_core.append({
            "xTloc": xloc.astype(np.float16).reshape(2, 128, NLOC),
            "W1e": W1e,
            "W2e": W2e,
            "b1": np.asarray(b1, np.float32)[None, :],
            "b2": np.asarray(b2, np.float32)[None, :],
            "hidx": hw[c],
            "adix": aw[c],
            "drel": dw[c],
        })
    return per_core


def _dma_gather_small(g, out_ap, in_ap, idxs_ap, num_idxs, elem_size,
                      elem_step, queue_num=0):
    """dma_gather with an arbitrary (<256B) payload. Only the row STRIDE must
    be a 256B multiple on the Q7 side; bass's elem_size%256 assert is
    over-conservative for the non-transpose path, so build the instruction
    directly."""
    import concourse.mybir as mybir
    stride_bytes = elem_step * mybir.dt.size(in_ap.dtype)
    assert stride_bytes % 256 == 0
    _in_ap = g.lower_ap_dma(in_ap, for_custom_bir_dma=True)
    _idxs_ap = g.lower_ap(idxs_ap)
    _out_ap = g.lower_ap(out_ap)
    return g.add_instruction(mybir.InstDMAGatherAnt(
        name=g.bass.get_next_instruction_name(),
        ins=[*_in_ap, _idxs_ap, g.lower_val_access(g.to_reg(num_idxs))],
        outs=[_out_ap],
        transpose=False,
        num_idxs=num_idxs,
        elem_size=elem_size,
        stride_bytes_256=stride_bytes // 256,
        gen_mode=0,
        single_packet=True,
        queue_num=queue_num,
        sbuf_tokens_per_rank=0,
        sbuf_free_dim_per_rank=0,
        sbuf_free_dim_pad_per_rank=0,
        sbuf_byte_offset=0,
    ))


def _build_nc():
    import concourse.bass as bass
    import concourse.bacc as bacc
    import concourse.mybir as mybir
    import concourse.tile as tile
    from concourse.library_config import mlp
    from concourse.masks import make_identity

    f32, f16, bf16, i16 = (mybir.dt.float32, mybir.dt.float16,
                           mybir.dt.bfloat16, mybir.dt.int16)
    AF = mybir.ActivationFunctionType
    OP = mybir.AluOpType

    nc = bacc.Bacc("TRN2", target_bir_lowering=False, debug=False,
                   num_devices=8, num_swdge_queues=4)

    xTloc = nc.dram_tensor("xTloc", [2, 128, NLOC], f16, kind="ExternalInput")
    W1e = nc.dram_tensor("W1e", [2, 128, 80], f16, kind="ExternalInput")
    W2e = nc.dram_tensor("W2e", [64, 42], f16, kind="ExternalInput")
    b1 = nc.dram_tensor("b1", [1, 64], f32, kind="ExternalInput")
    b2 = nc.dram_tensor("b2", [1, 40], f32, kind="ExternalInput")
    hidx = nc.dram_tensor("hidx", [128, NSLOT // 16], i16, kind="ExternalInput")
    adix = nc.dram_tensor("adix", [128, NSLOT // 16], i16, kind="ExternalInput")
    drel = nc.dram_tensor("drel", [128, NSLOT // 128], bf16, kind="ExternalInput")
    out = nc.dram_tensor("out", [NLOC, 40], f32, kind="ExternalOutput")

    tabin = nc.dram_tensor("tabin", [NLOC, 128], f16)    # local [h|1|as] rows
    taba = nc.dram_tensor("taba", [NLOC, 128], f16)      # ad1 cols 0:8, ad2 col 8
    tab2in = nc.dram_tensor("tab2in", [NLOC, 128], f16)  # local [h2|1|as2] rows
    tab1 = nc.dram_tensor("tab1", [NTOT, 128], f16, addr_space="Shared")
    tab2 = nc.dram_tensor("tab2", [NTOT, 128], f16, addr_space="Shared")

    def BC(ap, dims):
        return bass.AP(ap.tensor, ap.offset, dims)

    with tile.TileContext(nc) as tc:
        with tc.tile_pool(name="const", bufs=1) as pc:
            nc.gpsimd.load_library(mlp)

            adix_sb = pc.tile([128, NSLOT // 16], i16)
            drel_sb = pc.tile([128, NSLOT // 128], bf16)
            nc.sync.dma_start(adix_sb[:], adix[:])
            nc.sync.dma_start(drel_sb[:], drel[:])
            w1_sb = pc.tile([128, 2, 80], f16)
            nc.sync.dma_start(w1_sb[:], W1e[:].rearrange("k p n -> p k n"))
            w2_sb = pc.tile([64, 42], f16)
            nc.sync.dma_start(w2_sb[:], W2e[:])

            iota_i = pc.tile([128, 128], i16)
            nc.gpsimd.iota(iota_i[:], pattern=[[1, 128]], base=0,
                           channel_multiplier=0)
            iota_sb = pc.tile([128, 128], bf16)
            nc.vector.tensor_copy(out=iota_sb[:], in_=iota_i[:])
            # iotaT5[p, d*5+j] = d
            iotaT5 = pc.tile([128, 128, 5], bf16)
            nc.vector.tensor_copy(
                out=iotaT5[:],
                in_=BC(iota_sb[:], [iota_sb[:].ap[0], [1, 128], [0, 5]]))

            ident = pc.tile([128, 128], f16)
            make_identity(nc, ident[:])

            ones32 = pc.tile([1, 128], f32)
            nc.vector.memset(ones32[:], 1.0)

            b1r = pc.tile([128, 64], f32)
            b2r = pc.tile([128, 40], f32)
            with tc.tile_pool(name="pini", bufs=2, space="PSUM") as ppi:
                for row_d, width, dest in ((b1, 64, b1r), (b2, 40, b2r)):
                    t = pc.tile([1, width], f32, tag=f"rrow{width}")
                    nc.sync.dma_start(t[:], row_d[:])
                    ps = ppi.tile([128, width], f32, tag="rep")
                    nc.tensor.matmul(ps[:], lhsT=ones32[:], rhs=t[:],
                                     start=True, stop=True)
                    nc.vector.tensor_copy(out=dest[:], in_=ps[:])

            # ---------- phase A: sliced node pass -> tabin, taba ----------
            with (tc.tile_pool(name="pa", bufs=3) as pa,
                  tc.tile_pool(name="ppa", bufs=2, space="PSUM") as ppa):
                for i0 in range(0, NW, 4):
                    xt = pa.tile([128, 2, 512], f16, tag="xt")
                    for k in range(2):
                        nc.sync.dma_start(
                            xt[:, k, :], xTloc[k, :, i0 * 128:(i0 + 4) * 128])
                    slab = pa.tile([128, 4, 80], f16, tag="slab")
                    nc.vector.memset(slab[:], 1.0)
                    ad4 = pa.tile([128, 4, 8], f16, tag="ad4")
                    for j in range(4):
                        ps = ppa.tile([128, 80], f32, tag="np1")
                        for k in range(2):
                            nc.tensor.matmul(
                                ps[:], lhsT=xt[:, k, j * 128:(j + 1) * 128],
                                rhs=w1_sb[:, k, :], start=(k == 0),
                                stop=(k == 1))
                        sj = slab[:, j, :]
                        nc.vector.tensor_copy(
                            out=BC(sj, [sj.ap[0], [9, 8], [1, 8]]),
                            in_=ps[:, 0:64])
                        nc.vector.tensor_copy(out=slab[:, j, 72:80],
                                              in_=ps[:, 64:72])
                        nc.vector.tensor_copy(out=ad4[:, j, :],
                                              in_=ps[:, 72:80])
                    nc.sync.dma_start(
                        tabin[i0 * 128:(i0 + 4) * 128, 0:80].rearrange(
                            "(a b) c -> b a c", b=128),
                        slab[:])
                    nc.sync.dma_start(
                        taba[i0 * 128:(i0 + 4) * 128, 0:8].rearrange(
                            "(a b) c -> b a c", b=128),
                        ad4[:])

            # ---------- AllGather layer-1 table ----------
            nc.gpsimd.collective_compute(
                "AllGather", OP.bypass, ins=[tabin[:]], outs=[tab1[:]],
                replica_groups=[list(range(8))])

            # ---------- phase B: layer-1 edge pass ----------
            with (tc.tile_pool(name="pat", bufs=1) as pat,
                  tc.tile_pool(name="pb", bufs=2) as pb,
                  tc.tile_pool(name="pw", bufs=3) as pw,
                  tc.tile_pool(name="ppb", bufs=2, space="PSUM") as ppb):
                # per-slot alpha_dst for all blocks (overlaps the AllGather)
                at_t = []
                for b in range(NBLK):
                    at = pat.tile([128, NQ * B * 5, 8], f16, tag=f"at{b}")
                    _dma_gather_small(
                        nc.gpsimd, at[:], taba[:, 0:8],
                        adix_sb[:, b * 1600:(b + 1) * 1600],
                        NQ * B * SQ, 8, 128, queue_num=b % 4)
                    at_t.append(at)

                for b in range(NBLK):
                    hix = pb.tile([128, 1600], i16, tag="hix")
                    nc.sync.dma_start(hix[:],
                                      hidx[:, b * 1600:(b + 1) * 1600])
                    ht = pb.tile([128, NQ, B * 5, 80], f16, tag="ht")
                    for qi in range(NQ):
                        _dma_gather_small(
                            nc.gpsimd, ht[:, qi], tab1[qi * QS:(qi + 1) * QS, 0:80],
                            hix[:, qi * 400:(qi + 1) * 400],
                            B * SQ, 80, 128, queue_num=qi)
                    at = at_t[b]

                    for wi in range(B):
                        w = b * B + wi
                        # one-hot (slots x dst_rel), [128, 128, NQ, 5]
                        oh = pw.tile([128, 128, NQ, 5], bf16, tag="oh")
                        dr = drel_sb[:, b * 200 + wi * 5:(b + 1) * 200]
                        nc.vector.tensor_tensor(
                            out=oh[:],
                            in0=BC(dr, [dr.ap[0], [0, 128], [50, NQ], [1, 5]]),
                            in1=BC(iotaT5[:],
                                   [iotaT5[:].ap[0], [5, 128], [0, NQ], [1, 5]]),
                            op=OP.is_equal)

                        e1 = pw.tile([128, NQ, 5, 8], f16, tag="e1")
                        hslice = ht[:, :, wi * 5:(wi + 1) * 5, 72:80]
                        aslice = at[:, wi * 5:, :]
                        nc.vector.tensor_tensor(
                            out=e1[:], in0=hslice,
                            in1=BC(aslice, [aslice.ap[0], [400, NQ], [8, 5], [1, 8]]),
                            op=OP.add)
                        lr = pw.tile([128, NQ, 5, 8], f16, tag="lr")
                        nc.scalar.activation(out=lr[:], in_=e1[:],
                                             func=AF.Prelu, alpha=0.2)
                        wg = pw.tile([128, NQ, 5, 8], f32, tag="wg")
                        nc.scalar.activation(out=wg[:], in_=lr[:], func=AF.Exp)

                        # msg[s, q, j, h*9+k] = ht * wg (k=8 col gives w itself)
                        msg = pw.tile([128, NQ, 5, 72], bf16, tag="msg")
                        for qi in range(NQ):
                            mq = msg[:, qi]
                            hq = ht[:, qi, wi * 5:(wi + 1) * 5, 0:72]
                            wq = wg[:, qi]
                            nc.vector.tensor_tensor(
                                out=BC(mq, [mq.ap[0], [72, 5], [9, 8], [1, 9]]),
                                in0=BC(hq, [hq.ap[0], [80, 5], [9, 8], [1, 9]]),
                                in1=BC(wq, [wq.ap[0], [8, 5], [1, 8], [0, 9]]),
                                op=OP.mult)

                        ps = ppb.tile([128, 72], f32, tag="agg")
                        for k in range(NQ * 5):
                            qi, j = divmod(k, 5)
                            nc.tensor.matmul(ps[:], lhsT=oh[:, :, qi, j],
                                             rhs=msg[:, qi, j, :],
                                             start=(k == 0),
                                             stop=(k == NQ * 5 - 1))

                        den = pw.tile([128, 8], f32, tag="den")
                        pden = ps[:, 8:72]
                        nc.vector.tensor_scalar_add(
                            den[:], BC(pden, [pden.ap[0], [9, 8]]), ACC_EPS)
                        rec = pw.tile([128, 8], f32, tag="rec")
                        nc.vector.reciprocal(rec[:], den[:])
                        o1 = pw.tile([128, 64], f32, tag="o1")
                        pnum = ps[:, 0:72]
                        rfull = rec[:]
                        nc.vector.tensor_tensor(
                            out=BC(o1[:], [o1[:].ap[0], [8, 8], [1, 8]]),
                            in0=BC(pnum, [pnum.ap[0], [9, 8], [1, 8]]),
                            in1=BC(rfull, [rfull.ap[0], [1, 8], [0, 8]]),
                            op=OP.mult)
                        o1b = pw.tile([128, 64], f32, tag="o1b")
                        nc.vector.tensor_tensor(out=o1b[:], in0=o1[:],
                                                in1=b1r[:], op=OP.add)
                        # elu = relu(x) + exp(-relu(-x)) - 1
                        u = pw.tile([128, 64], f32, tag="u")
                        nc.scalar.activation(out=u[:], in_=o1b[:],
                                             func=AF.Relu, scale=-1.0)
                        t2 = pw.tile([128, 64], f32, tag="t2")
                        nc.scalar.activation(out=t2[:], in_=u[:],
                                             func=AF.Exp, scale=-1.0)
                        t3 = pw.tile([128, 64], f32, tag="t3")
                        nc.scalar.activation(out=t3[:], in_=o1b[:], func=AF.Relu)
                        t4 = pw.tile([128, 64], f32, tag="t4")
                        nc.vector.tensor_tensor(out=t4[:], in0=t2[:], in1=t3[:],
                                                op=OP.add)
                        hl16 = pw.tile([128, 64], f16, tag="hl16")
                        nc.vector.tensor_scalar_add(hl16[:], t4[:], -1.0)

                        pst = ppb.tile([64, 128], f16, tag="tr")
                        nc.tensor.transpose(out=pst[:], in_=hl16[:],
                                            identity=ident[:])
                        hlT = pw.tile([64, 128], f16, tag="hlT")
                        nc.scalar.copy(out=hlT[:], in_=pst[:])
                        ps2 = ppb.tile([128, 42], f32, tag="mm2")
                        nc.tensor.matmul(ps2[:], lhsT=hlT[:], rhs=w2_sb[:],
                                         start=True, stop=True)
                        t2row = pw.tile([128, 42], f16, tag="t2row")
                        nc.vector.memset(t2row[:, 40:41], 1.0)
                        nc.scalar.copy(out=t2row[:, 0:40], in_=ps2[:, 0:40])
                        nc.scalar.copy(out=t2row[:, 41:42], in_=ps2[:, 40:41])
                        ad2t = pw.tile([128, 1], f16, tag="ad2t")
                        nc.scalar.copy(out=ad2t[:], in_=ps2[:, 41:42])
                        nc.sync.dma_start(
                            tab2in[w * 128:(w + 1) * 128, 0:42], t2row[:])
                        nc.sync.dma_start(
                            taba[w * 128:(w + 1) * 128, 8:9], ad2t[:])

            # ---------- AllGather layer-2 table ----------
            nc.gpsimd.collective_compute(
                "AllGather", OP.bypass, ins=[tab2in[:]], outs=[tab2[:]],
                replica_groups=[list(range(8))])

            # ---------- phase D: layer-2 edge pass -> out ----------
            with (tc.tile_pool(name="pat2", bufs=1) as pat2,
                  tc.tile_pool(name="pd", bufs=2) as pd,
                  tc.tile_pool(name="pw2", bufs=3) as pw2,
                  tc.tile_pool(name="ppd", bufs=2, space="PSUM") as ppd):
                at2_t = []
                for b in range(NBLK):
                    at2 = pat2.tile([128, NQ * B * 5, 1], f16, tag=f"at2{b}")
                    _dma_gather_small(
                        nc.gpsimd, at2[:], taba[:, 8:9],
                        adix_sb[:, b * 1600:(b + 1) * 1600],
                        NQ * B * SQ, 1, 128, queue_num=b % 4)
                    at2_t.append(at2)

                for b in range(NBLK):
                    hix = pd.tile([128, 1600], i16, tag="hix2")
                    nc.sync.dma_start(hix[:],
                                      hidx[:, b * 1600:(b + 1) * 1600])
                    g2 = pd.tile([128, NQ, B * 5, 42], f16, tag="g2")
                    for qi in range(NQ):
                        _dma_gather_small(
                            nc.gpsimd, g2[:, qi], tab2[qi * QS:(qi + 1) * QS, 0:42],
                            hix[:, qi * 400:(qi + 1) * 400],
                            B * SQ, 42, 128, queue_num=qi)
                    at2 = at2_t[b]

                    for wi in range(B):
                        w = b * B + wi
                        oh = pw2.tile([128, 128, NQ, 5], bf16, tag="oh2")
                        dr = drel_sb[:, b * 200 + wi * 5:(b + 1) * 200]
                        nc.vector.tensor_tensor(
                            out=oh[:],
                            in0=BC(dr, [dr.ap[0], [0, 128], [50, NQ], [1, 5]]),
                            in1=BC(iotaT5[:],
                                   [iotaT5[:].ap[0], [5, 128], [0, NQ], [1, 5]]),
                            op=OP.is_equal)

                        e2 = pw2.tile([128, NQ, 5], f32, tag="e2")
                        gsl = g2[:, :, wi * 5:(wi + 1) * 5, 41:42]
                        asl = at2[:, wi * 5:, :]
                        nc.vector.tensor_tensor(
                            out=e2[:],
                            in0=BC(gsl, [gsl.ap[0], [2100, NQ], [42, 5]]),
                            in1=BC(asl, [asl.ap[0], [50, NQ], [1, 5]]),
                            op=OP.add)
                        lr2 = pw2.tile([128, NQ, 5], f32, tag="lr2")
                        nc.scalar.activation(out=lr2[:], in_=e2[:],
                                             func=AF.Prelu, alpha=0.2)
                        wg2 = pw2.tile([128, NQ, 5], f32, tag="wg2")
                        nc.scalar.activation(out=wg2[:], in_=lr2[:], func=AF.Exp)

                        m2 = pw2.tile([128, NQ, 5, 41], bf16, tag="m2")
                        g2w = g2[:, :, wi * 5:(wi + 1) * 5, 0:41]
                        w2f = wg2[:]
                        nc.vector.tensor_tensor(
                            out=m2[:], in0=g2w,
                            in1=BC(w2f, [w2f.ap[0], [5, NQ], [1, 5], [0, 41]]),
                            op=OP.mult)

                        ps = ppd.tile([128, 41], f32, tag="agg2")
                        for k in range(NQ * 5):
                            qi, j = divmod(k, 5)
                            nc.tensor.matmul(ps[:], lhsT=oh[:, :, qi, j],
                                             rhs=m2[:, qi, j, :],
                                             start=(k == 0),
                                             stop=(k == NQ * 5 - 1))

                        den2 = pw2.tile([128, 1], f32, tag="den2")
                        nc.vector.tensor_scalar_add(den2[:], ps[:, 40:41],
                                                    ACC_EPS)
                        rec2 = pw2.tile([128, 1], f32, tag="rec2")
                        nc.vector.reciprocal(rec2[:], den2[:])
                        o2 = pw2.tile([128, 40], f32, tag="o2")
                        nc.vector.tensor_scalar_mul(o2[:], ps[:, 0:40], rec2[:])
                        o2b = pw2.tile([128, 40], f32, tag="o2b")
                        nc.vector.tensor_tensor(out=o2b[:], in0=o2[:],
                                                in1=b2r[:], op=OP.add)
                        nc.sync.dma_start(out[w * 128:(w + 1) * 128, :], o2b[:])

    nc.finalize()
    return nc


def kernel(**inputs):
    per_core = _host_prep(**inputs)
    if "nc" not in _CACHE:
        _CACHE["nc"] = _build_nc()
    nc = _CACHE["nc"]
    from concourse.bass_utils import run_bass_kernel_spmd
    res = run_bass_kernel_spmd(nc, per_core, list(range(8)))
    full = np.concatenate([res.results[c]["out"] for c in range(8)], axis=0)
    return np.ascontiguousarray(full[:N]).astype(np.float32)


# revision 12
# speedup vs baseline: 2.2960x; 1.1007x over previous
"""GAT (2-layer PyG GATConv, eval) on 8 Trainium2 NeuronCores — v2.

Sharding: nodes range-partitioned (NLOC=12800/core); each core owns edges whose
dst is in its range. Edges grouped block-major: (block of B=10 windows,
src-quadrant, window, 640-slot group). Per block: 4 batched dma_gathers fetch
[h|1.0|as] rows (160B payload) from the layer table + 1 small gather fetches
per-slot alpha_dst (16B) from a local per-dst table — no per-chunk transpose
chains. Layer tables are built by a sliced node pass (1/8 nodes per core) and
AllGathered. Softmax uses the shift-invariant no-max form; the denominator
comes free from a 1.0 column folded into the gathered rows. Segment reduction
is a PE matmul with one-hot(dst_rel) per 128-slot chunk.
"""
import numpy as np
import ml_dtypes

N = 100000
E = 1600000
NF = 256
HEADS, NHID = 8, 8
NCLASS = 40
NLOC = 12800               # nodes per core
NW = 100                   # 128-dst windows per core
B = 10                     # windows per block
NBLK = NW // B             # 10 blocks per core
NQ = 4                     # src quadrants
QS = 25600                 # rows per quadrant sub-table
SQ = 640                   # edge slots per (window, quadrant) group
NSLOT = NBLK * NQ * B * SQ  # 256000 slots per core
NTOT = 102400
ACC_EPS = 1e-16

_CACHE = {}


def _host_prep(x, edge_index, W1, a1_src, a1_dst, b1, W2, a2_src, a2_dst, b2):
    src = np.asarray(edge_index[0], dtype=np.int64)
    dst = np.asarray(edge_index[1], dtype=np.int64)

    core = dst // NLOC
    dloc = dst - core * NLOC
    win = dloc >> 7
    q = src // QS
    blk = win // B
    wi = win - blk * B
    gid = ((core * NBLK + blk) * NQ + q) * B + wi
    order = np.argsort(gid, kind="stable")
    gsz = np.bincount(gid, minlength=8 * NBLK * NQ * B)
    assert gsz.max() <= SQ, f"group overflow: {gsz.max()} > {SQ}"
    starts = np.zeros_like(gsz)
    starts[1:] = np.cumsum(gsz)[:-1]
    g_sorted = gid[order]
    rank = np.arange(E) - starts[g_sorted]
    slot_global = g_sorted * SQ + rank
    cc = slot_global // NSLOT
    sc = slot_global - cc * NSLOT

    hidx = np.zeros((8, NSLOT), np.int16)          # pad -> row 0 of sub-table
    adix = np.zeros((8, NSLOT), np.int16)          # pad -> local row 0
    drel = np.full((8, NSLOT), 128.0, np.float32)  # pad -> out-of-window
    hidx[cc, sc] = (src[order] - q[order] * QS).astype(np.int16)
    adix[cc, sc] = dloc[order].astype(np.int16)
    drel[cc, sc] = (dloc[order] & 127).astype(np.float32)

    # dma_gather idx wrap: token s -> [s%16, s//16], replicated into all eight
    # 16-partition blocks (Q7 cpu pairs for the 4 SWDGE queues).
    slots = np.arange(NSLOT)

    def wrap16(a):
        w = np.zeros((8, 128, NSLOT // 16), np.int16)
        w[:, slots % 16, slots // 16] = a
        for r in range(1, 8):
            w[:, 16 * r:16 * (r + 1)] = w[:, :16]
        return w

    hw = wrap16(hidx)
    aw = wrap16(adix)

    # dst_rel, chunk-major: token s -> [s%128, s//128]
    dw = np.zeros((8, 128, NSLOT // 128), np.float32)
    dw[:, slots % 128, slots // 128] = drel
    dw = dw.astype(ml_dtypes.bfloat16)

    # fold attention vectors into the node-pass weights
    W1 = np.asarray(W1, np.float32)
    v_s1 = np.einsum("chk,hk->ch", W1.reshape(NF, HEADS, NHID),
                     np.asarray(a1_src, np.float32))
    v_d1 = np.einsum("chk,hk->ch", W1.reshape(NF, HEADS, NHID),
                     np.asarray(a1_dst, np.float32))
    W1e = np.concatenate([W1, v_s1, v_d1], axis=1).reshape(2, 128, 80)
    W1e = W1e.astype(np.float16)

    W2 = np.asarray(W2, np.float32)
    v_s2 = W2 @ np.asarray(a2_src, np.float32)[0]
    v_d2 = W2 @ np.asarray(a2_dst, np.float32)[0]
    W2e = np.concatenate([W2, v_s2[:, None], v_d2[:, None]],
                         axis=1).astype(np.float16)       # [64, 42]

    xp = np.zeros((NTOT, NF), np.float32)
    xp[:N] = np.asarray(x, np.float32)

    per_core = []
    for c in range(8):
        xloc = np.ascontiguousarray(xp[c * NLOC:(c + 1) * NLOC].T)
        per_core.append({
            "xTloc": xloc.astype(np.float16).reshape(2, 128, NLOC),
            "W1e": W1e,
            "W2e": W2e,
            "b1": np.asarray(b1, np.float32)[None, :],
            "b2": np.asarray(b2, np.float32)[None, :],
            "hidx": hw[c],
            "adix": aw[c],
            "drel": dw[c],
        })
    return per_core


def _dma_gather_small(g, out_ap, in_ap, idxs_ap, num_idxs, elem_size,
                      elem_step, queue_num=0):
    """dma_gather with an arbitrary (<256B) payload. Only the row STRIDE must
    be a 256B multiple on the Q7 side; bass's elem_size%256 assert is
    over-conservative for the non-transpose path, so build the instruction
    directly."""
    import concourse.mybir as mybir
    stride_bytes = elem_step * mybir.dt.size(in_ap.dtype)
    assert stride_bytes % 256 == 0
    _in_ap = g.lower_ap_dma(in_ap, for_custom_bir_dma=True)
    _idxs_ap = g.lower_ap(idxs_ap)
    _out_ap = g.lower_ap(out_ap)
    return g.add_instruction(mybir.InstDMAGatherAnt(
        name=g.bass.get_next_instruction_name(),
        ins=[*_in_ap, _idxs_ap, g.lower_val_access(g.to_reg(num_idxs))],
        outs=[_out_ap],
        transpose=False,
        num_idxs=num_idxs,
        elem_size=elem_size,
        stride_bytes_256=stride_bytes // 256,
        gen_mode=0,
        single_packet=True,
        queue_num=queue_num,
        sbuf_tokens_per_rank=0,
        sbuf_free_dim_per_rank=0,
        sbuf_free_dim_pad_per_rank=0,
        sbuf_byte_offset=0,
    ))


def _build_nc():
    import concourse.bass as bass
    import concourse.bacc as bacc
    import concourse.mybir as mybir
    import concourse.tile as tile
    from concourse.library_config import mlp
    from concourse.masks import make_identity

    f32, f16, bf16, i16 = (mybir.dt.float32, mybir.dt.float16,
                           mybir.dt.bfloat16, mybir.dt.int16)
    AF = mybir.ActivationFunctionType
    OP = mybir.AluOpType

    nc = bacc.Bacc("TRN2", target_bir_lowering=False, debug=False,
                   num_devices=8, num_swdge_queues=4)

    xTloc = nc.dram_tensor("xTloc", [2, 128, NLOC], f16, kind="ExternalInput")
    W1e = nc.dram_tensor("W1e", [2, 128, 80], f16, kind="ExternalInput")
    W2e = nc.dram_tensor("W2e", [64, 42], f16, kind="ExternalInput")
    b1 = nc.dram_tensor("b1", [1, 64], f32, kind="ExternalInput")
    b2 = nc.dram_tensor("b2", [1, 40], f32, kind="ExternalInput")
    hidx = nc.dram_tensor("hidx", [128, NSLOT // 16], i16, kind="ExternalInput")
    adix = nc.dram_tensor("adix", [128, NSLOT // 16], i16, kind="ExternalInput")
    drel = nc.dram_tensor("drel", [128, NSLOT // 128], bf16, kind="ExternalInput")
    out = nc.dram_tensor("out", [NLOC, 40], f32, kind="ExternalOutput")

    tabin = nc.dram_tensor("tabin", [NLOC, 128], f16)    # local [h|1|as] rows
    taba = nc.dram_tensor("taba", [NLOC, 128], f16)      # ad1 cols 0:8, ad2 col 8
    tab2in = nc.dram_tensor("tab2in", [NLOC, 128], f16)  # local [h2|1|as2] rows
    tab1 = nc.dram_tensor("tab1", [NTOT, 128], f16, addr_space="Shared")
    tab2 = nc.dram_tensor("tab2", [NTOT, 128], f16, addr_space="Shared")

    def BC(ap, dims):
        return bass.AP(ap.tensor, ap.offset, dims)

    with tile.TileContext(nc) as tc:
        with tc.tile_pool(name="const", bufs=1) as pc:
            nc.gpsimd.load_library(mlp)

            adix_sb = pc.tile([128, NSLOT // 16], i16)
            drel_sb = pc.tile([128, NSLOT // 128], bf16)
            nc.sync.dma_start(adix_sb[:], adix[:])
            nc.sync.dma_start(drel_sb[:], drel[:])
            w1_sb = pc.tile([128, 2, 80], f16)
            nc.sync.dma_start(w1_sb[:], W1e[:].rearrange("k p n -> p k n"))
            w2_sb = pc.tile([64, 42], f16)
            nc.sync.dma_start(w2_sb[:], W2e[:])

            iota_i = pc.tile([128, 128], i16)
            nc.gpsimd.iota(iota_i[:], pattern=[[1, 128]], base=0,
                           channel_multiplier=0)
            iota_sb = pc.tile([128, 128], bf16)
            nc.vector.tensor_copy(out=iota_sb[:], in_=iota_i[:])
            # iotaT5[p, d*5+j] = d
            iotaT5 = pc.tile([128, 128, 5], bf16)
            nc.vector.tensor_copy(
                out=iotaT5[:],
                in_=BC(iota_sb[:], [iota_sb[:].ap[0], [1, 128], [0, 5]]))

            ident = pc.tile([128, 128], f16)
            make_identity(nc, ident[:])

            ones32 = pc.tile([1, 128], f32)
            nc.vector.memset(ones32[:], 1.0)

            b1r = pc.tile([128, 64], f32)
            b2r = pc.tile([128, 40], f32)
            with tc.tile_pool(name="pini", bufs=2, space="PSUM") as ppi:
                for row_d, width, dest in ((b1, 64, b1r), (b2, 40, b2r)):
                    t = pc.tile([1, width], f32, tag=f"rrow{width}")
                    nc.sync.dma_start(t[:], row_d[:])
                    ps = ppi.tile([128, width], f32, tag="rep")
                    nc.tensor.matmul(ps[:], lhsT=ones32[:], rhs=t[:],
                                     start=True, stop=True)
                    nc.vector.tensor_copy(out=dest[:], in_=ps[:])

            # ---------- phase A: sliced node pass -> tabin, taba ----------
            with (tc.tile_pool(name="pa", bufs=3) as pa,
                  tc.tile_pool(name="ppa", bufs=2, space="PSUM") as ppa):
                for i0 in range(0, NW, 4):
                    xt = pa.tile([128, 2, 512], f16, tag="xt")
                    for k in range(2):
                        nc.sync.dma_start(
                            xt[:, k, :], xTloc[k, :, i0 * 128:(i0 + 4) * 128])
                    slab = pa.tile([128, 4, 80], f16, tag="slab")
                    nc.vector.memset(slab[:], 1.0)
                    ad4 = pa.tile([128, 4, 8], f16, tag="ad4")
                    for j in range(4):
                        ps = ppa.tile([128, 80], f32, tag="np1")
                        for k in range(2):
                            nc.tensor.matmul(
                                ps[:], lhsT=xt[:, k, j * 128:(j + 1) * 128],
                                rhs=w1_sb[:, k, :], start=(k == 0),
                                stop=(k == 1))
                        sj = slab[:, j, :]
                        nc.scalar.copy(
                            out=BC(sj, [sj.ap[0], [9, 8], [1, 8]]),
                            in_=ps[:, 0:64])
                        nc.scalar.copy(out=slab[:, j, 72:80],
                                       in_=ps[:, 64:72])
                        nc.scalar.copy(out=ad4[:, j, :],
                                       in_=ps[:, 72:80])
                    nc.sync.dma_start(
                        tabin[i0 * 128:(i0 + 4) * 128, 0:80].rearrange(
                            "(a b) c -> b a c", b=128),
                        slab[:])
                    nc.sync.dma_start(
                        taba[i0 * 128:(i0 + 4) * 128, 0:8].rearrange(
                            "(a b) c -> b a c", b=128),
                        ad4[:])

            # ---------- AllGather layer-1 table ----------
            nc.gpsimd.collective_compute(
                "AllGather", OP.bypass, ins=[tabin[:]], outs=[tab1[:]],
                replica_groups=[list(range(8))])

            # ---------- phase B: layer-1 edge pass ----------
            with (tc.tile_pool(name="pat", bufs=1) as pat,
                  tc.tile_pool(name="pb", bufs=2) as pb,
                  tc.tile_pool(name="pw", bufs=3) as pw,
                  tc.tile_pool(name="ppb", bufs=2, space="PSUM") as ppb):
                # per-slot alpha_dst for the whole phase (overlaps the AllGather)
                at = pat.tile([128, NSLOT // 128, 8], f16, tag="at")
                _dma_gather_small(nc.gpsimd, at[:], taba[:, 0:8],
                                  adix_sb[:], NSLOT, 8, 128, queue_num=0)

                for b in range(NBLK):
                    hix = pb.tile([128, 1600], i16, tag="hix")
                    nc.sync.dma_start(hix[:],
                                      hidx[:, b * 1600:(b + 1) * 1600])
                    ht = pb.tile([128, NQ, B * 5, 80], f16, tag="ht")
                    for qi in range(NQ):
                        _dma_gather_small(
                            nc.gpsimd, ht[:, qi], tab1[qi * QS:(qi + 1) * QS, 0:80],
                            hix[:, qi * 400:(qi + 1) * 400],
                            B * SQ, 80, 128, queue_num=qi)

                    for wi in range(B):
                        w = b * B + wi
                        # one-hot (slots x dst_rel), [128, 128, NQ, 5]
                        oh = pw.tile([128, 128, NQ, 5], bf16, tag="oh")
                        dr = drel_sb[:, b * 200 + wi * 5:(b + 1) * 200]
                        nc.vector.tensor_tensor(
                            out=oh[:],
                            in0=BC(dr, [dr.ap[0], [0, 128], [50, NQ], [1, 5]]),
                            in1=BC(iotaT5[:],
                                   [iotaT5[:].ap[0], [5, 128], [0, NQ], [1, 5]]),
                            op=OP.is_equal)

                        e1 = pw.tile([128, NQ, 5, 8], f16, tag="e1")
                        hslice = ht[:, :, wi * 5:(wi + 1) * 5, 72:80]
                        aslice = at[:, b * 200 + wi * 5:, :]
                        nc.vector.tensor_tensor(
                            out=e1[:], in0=hslice,
                            in1=BC(aslice, [aslice.ap[0], [400, NQ], [8, 5], [1, 8]]),
                            op=OP.add)
                        lr = pw.tile([128, NQ, 5, 8], f16, tag="lr")
                        nc.scalar.activation(out=lr[:], in_=e1[:],
                                             func=AF.Prelu, alpha=0.2)
                        # wgx[s, q, j, h*9+k] = exp(lr[s, q, j, h]) for all k —
                        # 9-expanded on ACT so the msg mult runs packed 2x
                        wgx = pw.tile([128, NQ, 5, 72], bf16, tag="wgx")
                        for qi in range(NQ):
                            wq = wgx[:, qi]
                            lq = lr[:, qi]
                            nc.scalar.activation(
                                out=BC(wq, [wq.ap[0], [72, 5], [9, 8], [1, 9]]),
                                in_=BC(lq, [lq.ap[0], [8, 5], [1, 8], [0, 9]]),
                                func=AF.Exp)

                        # msg[s, q, j, h*9+k] = ht * w (k=8 col gives w itself)
                        msg = pw.tile([128, NQ, 5, 72], bf16, tag="msg")
                        nc.vector.tensor_tensor(
                            out=msg[:],
                            in0=ht[:, :, wi * 5:(wi + 1) * 5, 0:72],
                            in1=wgx[:], op=OP.mult)

                        ps = ppb.tile([128, 72], f32, tag="agg")
                        for k in range(NQ * 5):
                            qi, j = divmod(k, 5)
                            nc.tensor.matmul(ps[:], lhsT=oh[:, :, qi, j],
                                             rhs=msg[:, qi, j, :],
                                             start=(k == 0),
                                             stop=(k == NQ * 5 - 1))

                        den = pw.tile([128, 8], f32, tag="den")
                        pden = ps[:, 8:72]
                        nc.vector.tensor_scalar_add(
                            den[:], BC(pden, [pden.ap[0], [9, 8]]), ACC_EPS)
                        rec = pw.tile([128, 8], f32, tag="rec")
                        nc.vector.reciprocal(rec[:], den[:])
                        o1 = pw.tile([128, 64], f32, tag="o1")
                        pnum = ps[:, 0:72]
                        rfull = rec[:]
                        nc.vector.tensor_tensor(
                            out=BC(o1[:], [o1[:].ap[0], [8, 8], [1, 8]]),
                            in0=BC(pnum, [pnum.ap[0], [9, 8], [1, 8]]),
                            in1=BC(rfull, [rfull.ap[0], [1, 8], [0, 8]]),
                            op=OP.mult)
                        o1b = pw.tile([128, 64], f32, tag="o1b")
                        nc.vector.tensor_tensor(out=o1b[:], in0=o1[:],
                                                in1=b1r[:], op=OP.add)
                        # elu = relu(x) + exp(min(x,0)) - 1
                        t1 = pw.tile([128, 64], f32, tag="t1")
                        nc.vector.tensor_scalar_min(t1[:], o1b[:], 0.0)
                        t2 = pw.tile([128, 64], f32, tag="t2")
                        nc.scalar.activation(out=t2[:], in_=t1[:], func=AF.Exp)
                        t3 = pw.tile([128, 64], f32, tag="t3")
                        nc.vector.tensor_scalar_max(t3[:], o1b[:], 0.0)
                        t4 = pw.tile([128, 64], f32, tag="t4")
                        nc.vector.tensor_tensor(out=t4[:], in0=t2[:], in1=t3[:],
                                                op=OP.add)
                        hl16 = pw.tile([128, 64], f16, tag="hl16")
                        nc.vector.tensor_scalar_add(hl16[:], t4[:], -1.0)

                        pst = ppb.tile([64, 128], f16, tag="tr")
                        nc.tensor.transpose(out=pst[:], in_=hl16[:],
                                            identity=ident[:])
                        hlT = pw.tile([64, 128], f16, tag="hlT")
                        nc.scalar.copy(out=hlT[:], in_=pst[:])
                        ps2 = ppb.tile([128, 42], f32, tag="mm2")
                        nc.tensor.matmul(ps2[:], lhsT=hlT[:], rhs=w2_sb[:],
                                         start=True, stop=True)
                        t2row = pw.tile([128, 42], f16, tag="t2row")
                        if w < 3:
                            nc.vector.memset(t2row[:, 40:41], 1.0)
                        nc.scalar.copy(out=t2row[:, 0:40], in_=ps2[:, 0:40])
                        nc.scalar.copy(out=t2row[:, 41:42], in_=ps2[:, 40:41])
                        ad2t = pw.tile([128, 1], f16, tag="ad2t")
                        nc.scalar.copy(out=ad2t[:], in_=ps2[:, 41:42])
                        nc.sync.dma_start(
                            tab2in[w * 128:(w + 1) * 128, 0:42], t2row[:])
                        nc.sync.dma_start(
                            taba[w * 128:(w + 1) * 128, 8:9], ad2t[:])

            # ---------- AllGather layer-2 table ----------
            nc.gpsimd.collective_compute(
                "AllGather", OP.bypass, ins=[tab2in[:]], outs=[tab2[:]],
                replica_groups=[list(range(8))])

            # ---------- phase D: layer-2 edge pass -> out ----------
            with (tc.tile_pool(name="pat2", bufs=1) as pat2,
                  tc.tile_pool(name="pd", bufs=2) as pd,
                  tc.tile_pool(name="pw2", bufs=3) as pw2,
                  tc.tile_pool(name="ppd", bufs=2, space="PSUM") as ppd):
                at2 = pat2.tile([128, NSLOT // 128, 1], f16, tag="at2")
                _dma_gather_small(nc.gpsimd, at2[:], taba[:, 8:9],
                                  adix_sb[:], NSLOT, 1, 128, queue_num=0)

                for b in range(NBLK):
                    hix = pd.tile([128, 1600], i16, tag="hix2")
                    nc.sync.dma_start(hix[:],
                                      hidx[:, b * 1600:(b + 1) * 1600])
                    g2 = pd.tile([128, NQ, B * 5, 42], f16, tag="g2")
                    for qi in range(NQ):
                        _dma_gather_small(
                            nc.gpsimd, g2[:, qi], tab2[qi * QS:(qi + 1) * QS, 0:42],
                            hix[:, qi * 400:(qi + 1) * 400],
                            B * SQ, 42, 128, queue_num=qi)

                    for wi in range(B):
                        w = b * B + wi
                        oh = pw2.tile([128, 128, NQ, 5], bf16, tag="oh2")
                        dr = drel_sb[:, b * 200 + wi * 5:(b + 1) * 200]
                        nc.vector.tensor_tensor(
                            out=oh[:],
                            in0=BC(dr, [dr.ap[0], [0, 128], [50, NQ], [1, 5]]),
                            in1=BC(iotaT5[:],
                                   [iotaT5[:].ap[0], [5, 128], [0, NQ], [1, 5]]),
                            op=OP.is_equal)

                        e2 = pw2.tile([128, NQ, 5], f32, tag="e2")
                        gsl = g2[:, :, wi * 5:(wi + 1) * 5, 41:42]
                        asl = at2[:, b * 200 + wi * 5:, :]
                        nc.vector.tensor_tensor(
                            out=e2[:],
                            in0=BC(gsl, [gsl.ap[0], [2100, NQ], [42, 5]]),
                            in1=BC(asl, [asl.ap[0], [50, NQ], [1, 5]]),
                            op=OP.add)
                        lr2 = pw2.tile([128, NQ, 5], f32, tag="lr2")
                        nc.scalar.activation(out=lr2[:], in_=e2[:],
                                             func=AF.Prelu, alpha=0.2)
                        # 41-expanded exp on ACT so the m2 mult runs packed 2x
                        wgx2 = pw2.tile([128, NQ, 5, 41], bf16, tag="wgx2")
                        l2f = lr2[:]
                        nc.scalar.activation(
                            out=wgx2[:],
                            in_=BC(l2f, [l2f.ap[0], [5, NQ], [1, 5], [0, 41]]),
                            func=AF.Exp)

                        m2 = pw2.tile([128, NQ, 5, 41], bf16, tag="m2")
                        nc.vector.tensor_tensor(
                            out=m2[:], in0=g2[:, :, wi * 5:(wi + 1) * 5, 0:41],
                            in1=wgx2[:], op=OP.mult)

                        ps = ppd.tile([128, 41], f32, tag="agg2")
                        for k in range(NQ * 5):
                            qi, j = divmod(k, 5)
                            nc.tensor.matmul(ps[:], lhsT=oh[:, :, qi, j],
                                             rhs=m2[:, qi, j, :],
                                             start=(k == 0),
                                             stop=(k == NQ * 5 - 1))

                        den2 = pw2.tile([128, 1], f32, tag="den2")
                        nc.vector.tensor_scalar_add(den2[:], ps[:, 40:41],
                                                    ACC_EPS)
                        rec2 = pw2.tile([128, 1], f32, tag="rec2")
                        nc.vector.reciprocal(rec2[:], den2[:])
                        o2 = pw2.tile([128, 40], f32, tag="o2")
                        nc.vector.tensor_scalar_mul(o2[:], ps[:, 0:40], rec2[:])
                        o2b = pw2.tile([128, 40], f32, tag="o2b")
                        nc.vector.tensor_tensor(out=o2b[:], in0=o2[:],
                                                in1=b2r[:], op=OP.add)
                        nc.sync.dma_start(out[w * 128:(w + 1) * 128, :], o2b[:])

    nc.finalize()
    return nc


def kernel(**inputs):
    per_core = _host_prep(**inputs)
    if "nc" not in _CACHE:
        _CACHE["nc"] = _build_nc()
    nc = _CACHE["nc"]
    from concourse.bass_utils import run_bass_kernel_spmd
    res = run_bass_kernel_spmd(nc, per_core, list(range(8)))
    full = np.concatenate([res.results[c]["out"] for c in range(8)], axis=0)
    return np.ascontiguousarray(full[:N]).astype(np.float32)


# revision 13
# speedup vs baseline: 2.3166x; 1.0089x over previous
"""GAT (2-layer PyG GATConv, eval) on 8 Trainium2 NeuronCores — v2.

Sharding: nodes range-partitioned (NLOC=12800/core); each core owns edges whose
dst is in its range. Edges grouped block-major: (block of B=10 windows,
src-quadrant, window, 640-slot group). Per block: 4 batched dma_gathers fetch
[h|1.0|as] rows (160B payload) from the layer table + 1 small gather fetches
per-slot alpha_dst (16B) from a local per-dst table — no per-chunk transpose
chains. Layer tables are built by a sliced node pass (1/8 nodes per core) and
AllGathered. Softmax uses the shift-invariant no-max form; the denominator
comes free from a 1.0 column folded into the gathered rows. Segment reduction
is a PE matmul with one-hot(dst_rel) per 128-slot chunk.
"""
import numpy as np
import ml_dtypes

N = 100000
E = 1600000
NF = 256
HEADS, NHID = 8, 8
NCLASS = 40
NLOC = 12800               # nodes per core
NW = 100                   # 128-dst windows per core
B = 10                     # windows per block
NBLK = NW // B             # 10 blocks per core
NQ = 4                     # src quadrants
QS = 25600                 # rows per quadrant sub-table
SQ = 640                   # edge slots per (window, quadrant) group
NSLOT = NBLK * NQ * B * SQ  # 256000 slots per core
NTOT = 102400
ACC_EPS = 1e-16

_CACHE = {}


def _host_prep(x, edge_index, W1, a1_src, a1_dst, b1, W2, a2_src, a2_dst, b2):
    src = np.asarray(edge_index[0], dtype=np.int64)
    dst = np.asarray(edge_index[1], dtype=np.int64)

    core = dst // NLOC
    dloc = dst - core * NLOC
    win = dloc >> 7
    q = src // QS
    blk = win // B
    wi = win - blk * B
    gid = ((core * NBLK + blk) * NQ + q) * B + wi
    order = np.argsort(gid, kind="stable")
    gsz = np.bincount(gid, minlength=8 * NBLK * NQ * B)
    assert gsz.max() <= SQ, f"group overflow: {gsz.max()} > {SQ}"
    starts = np.zeros_like(gsz)
    starts[1:] = np.cumsum(gsz)[:-1]
    g_sorted = gid[order]
    rank = np.arange(E) - starts[g_sorted]
    slot_global = g_sorted * SQ + rank
    cc = slot_global // NSLOT
    sc = slot_global - cc * NSLOT

    hidx = np.zeros((8, NSLOT), np.int16)          # pad -> row 0 of sub-table
    adix = np.zeros((8, NSLOT), np.int16)          # pad -> local row 0
    drel = np.full((8, NSLOT), 128.0, np.float32)  # pad -> out-of-window
    hidx[cc, sc] = (src[order] - q[order] * QS).astype(np.int16)
    adix[cc, sc] = dloc[order].astype(np.int16)
    drel[cc, sc] = (dloc[order] & 127).astype(np.float32)

    # dma_gather idx wrap: token s -> [s%16, s//16], replicated into all eight
    # 16-partition blocks (Q7 cpu pairs for the 4 SWDGE queues).
    slots = np.arange(NSLOT)

    def wrap16(a):
        w = np.zeros((8, 128, NSLOT // 16), np.int16)
        w[:, slots % 16, slots // 16] = a
        for r in range(1, 8):
            w[:, 16 * r:16 * (r + 1)] = w[:, :16]
        return w

    hw = wrap16(hidx)
    aw = wrap16(adix)

    # dst_rel, chunk-major: token s -> [s%128, s//128]
    dw = np.zeros((8, 128, NSLOT // 128), np.float32)
    dw[:, slots % 128, slots // 128] = drel
    dw = dw.astype(ml_dtypes.bfloat16)

    # fold attention vectors into the node-pass weights
    W1 = np.asarray(W1, np.float32)
    v_s1 = np.einsum("chk,hk->ch", W1.reshape(NF, HEADS, NHID),
                     np.asarray(a1_src, np.float32))
    v_d1 = np.einsum("chk,hk->ch", W1.reshape(NF, HEADS, NHID),
                     np.asarray(a1_dst, np.float32))
    W1e = np.concatenate([W1, v_s1, v_d1], axis=1).reshape(2, 128, 80)
    W1e = W1e.astype(np.float16)

    W2 = np.asarray(W2, np.float32)
    v_s2 = W2 @ np.asarray(a2_src, np.float32)[0]
    v_d2 = W2 @ np.asarray(a2_dst, np.float32)[0]
    W2e = np.concatenate([W2, v_s2[:, None], v_d2[:, None]],
                         axis=1).astype(np.float16)       # [64, 42]

    xp = np.zeros((NTOT, NF), np.float32)
    xp[:N] = np.asarray(x, np.float32)

    per_core = []
    for c in range(8):
        xloc = np.ascontiguousarray(xp[c * NLOC:(c + 1) * NLOC].T)
        per_core.append({
            "xTloc": xloc.astype(np.float16).reshape(2, 128, NLOC),
            "W1e": W1e,
            "W2e": W2e,
            "b1": np.asarray(b1, np.float32)[None, :],
            "b2": np.asarray(b2, np.float32)[None, :],
            "hidx": hw[c],
            "adix": aw[c],
            "drel": dw[c],
        })
    return per_core


def _dma_gather_small(g, out_ap, in_ap, idxs_ap, num_idxs, elem_size,
                      elem_step, queue_num=0):
    """dma_gather with an arbitrary (<256B) payload. Only the row STRIDE must
    be a 256B multiple on the Q7 side; bass's elem_size%256 assert is
    over-conservative for the non-transpose path, so build the instruction
    directly."""
    import concourse.mybir as mybir
    stride_bytes = elem_step * mybir.dt.size(in_ap.dtype)
    assert stride_bytes % 256 == 0
    _in_ap = g.lower_ap_dma(in_ap, for_custom_bir_dma=True)
    _idxs_ap = g.lower_ap(idxs_ap)
    _out_ap = g.lower_ap(out_ap)
    return g.add_instruction(mybir.InstDMAGatherAnt(
        name=g.bass.get_next_instruction_name(),
        ins=[*_in_ap, _idxs_ap, g.lower_val_access(g.to_reg(num_idxs))],
        outs=[_out_ap],
        transpose=False,
        num_idxs=num_idxs,
        elem_size=elem_size,
        stride_bytes_256=stride_bytes // 256,
        gen_mode=0,
        single_packet=True,
        queue_num=queue_num,
        sbuf_tokens_per_rank=0,
        sbuf_free_dim_per_rank=0,
        sbuf_free_dim_pad_per_rank=0,
        sbuf_byte_offset=0,
    ))


def _build_nc():
    import concourse.bass as bass
    import concourse.bacc as bacc
    import concourse.mybir as mybir
    import concourse.tile as tile
    from concourse.library_config import mlp
    from concourse.masks import make_identity

    f32, f16, bf16, i16 = (mybir.dt.float32, mybir.dt.float16,
                           mybir.dt.bfloat16, mybir.dt.int16)
    AF = mybir.ActivationFunctionType
    OP = mybir.AluOpType

    nc = bacc.Bacc("TRN2", target_bir_lowering=False, debug=False,
                   num_devices=8, num_swdge_queues=4)

    xTloc = nc.dram_tensor("xTloc", [2, 128, NLOC], f16, kind="ExternalInput")
    W1e = nc.dram_tensor("W1e", [2, 128, 80], f16, kind="ExternalInput")
    W2e = nc.dram_tensor("W2e", [64, 42], f16, kind="ExternalInput")
    b1 = nc.dram_tensor("b1", [1, 64], f32, kind="ExternalInput")
    b2 = nc.dram_tensor("b2", [1, 40], f32, kind="ExternalInput")
    hidx = nc.dram_tensor("hidx", [128, NSLOT // 16], i16, kind="ExternalInput")
    adix = nc.dram_tensor("adix", [128, NSLOT // 16], i16, kind="ExternalInput")
    drel = nc.dram_tensor("drel", [128, NSLOT // 128], bf16, kind="ExternalInput")
    out = nc.dram_tensor("out", [NLOC, 40], f32, kind="ExternalOutput")

    tabin = nc.dram_tensor("tabin", [NLOC, 128], f16)    # local [h|1|as] rows
    taba = nc.dram_tensor("taba", [NLOC, 128], f16)      # ad1 cols 0:8, ad2 col 8
    tab2in = nc.dram_tensor("tab2in", [NLOC, 128], f16)  # local [h2|1|as2] rows
    tab1 = nc.dram_tensor("tab1", [NTOT, 128], f16, addr_space="Shared")
    tab2 = nc.dram_tensor("tab2", [NTOT, 128], f16, addr_space="Shared")

    def BC(ap, dims):
        return bass.AP(ap.tensor, ap.offset, dims)

    with tile.TileContext(nc) as tc:
        with tc.tile_pool(name="const", bufs=1) as pc:
            nc.gpsimd.load_library(mlp)

            adix_sb = pc.tile([128, NSLOT // 16], i16)
            drel_sb = pc.tile([128, NSLOT // 128], bf16)
            nc.sync.dma_start(adix_sb[:], adix[:])
            nc.sync.dma_start(drel_sb[:], drel[:])
            w1_sb = pc.tile([128, 2, 80], f16)
            nc.sync.dma_start(w1_sb[:], W1e[:].rearrange("k p n -> p k n"))
            w2_sb = pc.tile([64, 42], f16)
            nc.sync.dma_start(w2_sb[:], W2e[:])

            iota_i = pc.tile([128, 128], i16)
            nc.gpsimd.iota(iota_i[:], pattern=[[1, 128]], base=0,
                           channel_multiplier=0)
            iota_sb = pc.tile([128, 128], bf16)
            nc.vector.tensor_copy(out=iota_sb[:], in_=iota_i[:])
            # iotaT5[p, d*5+j] = d
            iotaT5 = pc.tile([128, 128, 5], bf16)
            nc.vector.tensor_copy(
                out=iotaT5[:],
                in_=BC(iota_sb[:], [iota_sb[:].ap[0], [1, 128], [0, 5]]))

            ident = pc.tile([128, 128], f16)
            make_identity(nc, ident[:])

            ones32 = pc.tile([1, 128], f32)
            nc.vector.memset(ones32[:], 1.0)

            b1r = pc.tile([128, 64], f32)
            b2r = pc.tile([128, 40], f32)
            with tc.tile_pool(name="pini", bufs=2, space="PSUM") as ppi:
                for row_d, width, dest in ((b1, 64, b1r), (b2, 40, b2r)):
                    t = pc.tile([1, width], f32, tag=f"rrow{width}")
                    nc.sync.dma_start(t[:], row_d[:])
                    ps = ppi.tile([128, width], f32, tag="rep")
                    nc.tensor.matmul(ps[:], lhsT=ones32[:], rhs=t[:],
                                     start=True, stop=True)
                    nc.vector.tensor_copy(out=dest[:], in_=ps[:])

            # ---------- phase A: sliced node pass -> tabin, taba ----------
            with (tc.tile_pool(name="pa", bufs=3) as pa,
                  tc.tile_pool(name="ppa", bufs=2, space="PSUM") as ppa):
                for i0 in range(0, NW, 4):
                    xt = pa.tile([128, 2, 512], f16, tag="xt")
                    for k in range(2):
                        nc.sync.dma_start(
                            xt[:, k, :], xTloc[k, :, i0 * 128:(i0 + 4) * 128])
                    slab = pa.tile([128, 4, 88], f16, tag="slab")
                    nc.vector.memset(slab[:], 1.0)
                    for j in range(4):
                        ps = ppa.tile([128, 80], f32, tag="np1")
                        for k in range(2):
                            nc.tensor.matmul(
                                ps[:], lhsT=xt[:, k, j * 128:(j + 1) * 128],
                                rhs=w1_sb[:, k, :], start=(k == 0),
                                stop=(k == 1))
                        sj = slab[:, j, :]
                        nc.scalar.copy(
                            out=BC(sj, [sj.ap[0], [9, 8], [1, 8]]),
                            in_=ps[:, 0:64])
                        nc.vector.tensor_copy(out=slab[:, j, 72:88],
                                              in_=ps[:, 64:80])
                    nc.sync.dma_start(
                        tabin[i0 * 128:(i0 + 4) * 128, 0:80].rearrange(
                            "(a b) c -> b a c", b=128),
                        slab[:, :, 0:80])
                    nc.sync.dma_start(
                        taba[i0 * 128:(i0 + 4) * 128, 0:8].rearrange(
                            "(a b) c -> b a c", b=128),
                        slab[:, :, 80:88])

            # ---------- AllGather layer-1 table ----------
            nc.gpsimd.collective_compute(
                "AllGather", OP.bypass, ins=[tabin[:]], outs=[tab1[:]],
                replica_groups=[list(range(8))])

            # ---------- phase B: layer-1 edge pass ----------
            with (tc.tile_pool(name="pat", bufs=1) as pat,
                  tc.tile_pool(name="pb", bufs=2) as pb,
                  tc.tile_pool(name="pw", bufs=3) as pw,
                  tc.tile_pool(name="ppb", bufs=2, space="PSUM") as ppb):
                # per-slot alpha_dst for the whole phase (overlaps the AllGather)
                at = pat.tile([128, NSLOT // 128, 8], f16, tag="at")
                _dma_gather_small(nc.gpsimd, at[:], taba[:, 0:8],
                                  adix_sb[:], NSLOT, 8, 128, queue_num=0)

                for b in range(NBLK):
                    hix = pb.tile([128, 1600], i16, tag="hix")
                    nc.sync.dma_start(hix[:],
                                      hidx[:, b * 1600:(b + 1) * 1600])
                    ht = pb.tile([128, NQ, B * 5, 80], f16, tag="ht")
                    for qi in range(NQ):
                        _dma_gather_small(
                            nc.gpsimd, ht[:, qi], tab1[qi * QS:(qi + 1) * QS, 0:80],
                            hix[:, qi * 400:(qi + 1) * 400],
                            B * SQ, 80, 128, queue_num=qi)

                    for wi in range(B):
                        w = b * B + wi
                        # one-hot (slots x dst_rel), [128, 128, NQ, 5]
                        oh = pw.tile([128, 128, NQ, 5], bf16, tag="oh")
                        dr = drel_sb[:, b * 200 + wi * 5:(b + 1) * 200]
                        nc.vector.tensor_tensor(
                            out=oh[:],
                            in0=BC(dr, [dr.ap[0], [0, 128], [50, NQ], [1, 5]]),
                            in1=BC(iotaT5[:],
                                   [iotaT5[:].ap[0], [5, 128], [0, NQ], [1, 5]]),
                            op=OP.is_equal)

                        e1 = pw.tile([128, NQ, 5, 8], f16, tag="e1")
                        hslice = ht[:, :, wi * 5:(wi + 1) * 5, 72:80]
                        aslice = at[:, b * 200 + wi * 5:, :]
                        nc.vector.tensor_tensor(
                            out=e1[:], in0=hslice,
                            in1=BC(aslice, [aslice.ap[0], [400, NQ], [8, 5], [1, 8]]),
                            op=OP.add)
                        lr = pw.tile([128, NQ, 5, 8], f16, tag="lr")
                        nc.scalar.activation(out=lr[:], in_=e1[:],
                                             func=AF.Prelu, alpha=0.2)
                        # wgx[s, q, j, h*9+k] = exp(lr[s, q, j, h]) for all k —
                        # 9-expanded on ACT so the msg mult runs packed 2x
                        wgx = pw.tile([128, NQ, 5, 72], bf16, tag="wgx")
                        for qi in range(NQ):
                            wq = wgx[:, qi]
                            lq = lr[:, qi]
                            nc.scalar.activation(
                                out=BC(wq, [wq.ap[0], [72, 5], [9, 8], [1, 9]]),
                                in_=BC(lq, [lq.ap[0], [8, 5], [1, 8], [0, 9]]),
                                func=AF.Exp)

                        # msg[s, q, j, h*9+k] = ht * w (k=8 col gives w itself)
                        msg = pw.tile([128, NQ, 5, 72], bf16, tag="msg")
                        nc.vector.tensor_tensor(
                            out=msg[:],
                            in0=ht[:, :, wi * 5:(wi + 1) * 5, 0:72],
                            in1=wgx[:], op=OP.mult)

                        ps = ppb.tile([128, 72], f32, tag="agg")
                        for k in range(NQ * 5):
                            qi, j = divmod(k, 5)
                            nc.tensor.matmul(ps[:], lhsT=oh[:, :, qi, j],
                                             rhs=msg[:, qi, j, :],
                                             start=(k == 0),
                                             stop=(k == NQ * 5 - 1))

                        den = pw.tile([128, 8], f32, tag="den")
                        pden = ps[:, 8:72]
                        nc.vector.tensor_scalar_add(
                            den[:], BC(pden, [pden.ap[0], [9, 8]]), ACC_EPS)
                        rec = pw.tile([128, 8], f32, tag="rec")
                        nc.vector.reciprocal(rec[:], den[:])
                        o1 = pw.tile([128, 64], f32, tag="o1")
                        pnum = ps[:, 0:72]
                        rfull = rec[:]
                        nc.vector.tensor_tensor(
                            out=BC(o1[:], [o1[:].ap[0], [8, 8], [1, 8]]),
                            in0=BC(pnum, [pnum.ap[0], [9, 8], [1, 8]]),
                            in1=BC(rfull, [rfull.ap[0], [1, 8], [0, 8]]),
                            op=OP.mult)
                        o1b = pw.tile([128, 64], f32, tag="o1b")
                        nc.vector.tensor_tensor(out=o1b[:], in0=o1[:],
                                                in1=b1r[:], op=OP.add)
                        # elu = relu(x) + exp(min(x,0)) - 1
                        t1 = pw.tile([128, 64], f32, tag="t1")
                        nc.vector.tensor_scalar_min(t1[:], o1b[:], 0.0)
                        t2 = pw.tile([128, 64], f32, tag="t2")
                        nc.scalar.activation(out=t2[:], in_=t1[:], func=AF.Exp)
                        t3 = pw.tile([128, 64], f32, tag="t3")
                        nc.vector.tensor_scalar_max(t3[:], o1b[:], 0.0)
                        t4 = pw.tile([128, 64], f32, tag="t4")
                        nc.vector.tensor_tensor(out=t4[:], in0=t2[:], in1=t3[:],
                                                op=OP.add)
                        hl16 = pw.tile([128, 64], f16, tag="hl16")
                        nc.vector.tensor_scalar_add(hl16[:], t4[:], -1.0)

                        pst = ppb.tile([64, 128], f16, tag="tr")
                        nc.tensor.transpose(out=pst[:], in_=hl16[:],
                                            identity=ident[:])
                        hlT = pw.tile([64, 128], f16, tag="hlT")
                        nc.scalar.copy(out=hlT[:], in_=pst[:])
                        ps2 = ppb.tile([128, 42], f32, tag="mm2")
                        nc.tensor.matmul(ps2[:], lhsT=hlT[:], rhs=w2_sb[:],
                                         start=True, stop=True)
                        t2row = pw.tile([128, 42], f16, tag="t2row")
                        if w < 3:
                            nc.vector.memset(t2row[:, 40:41], 1.0)
                        nc.scalar.copy(out=t2row[:, 0:40], in_=ps2[:, 0:40])
                        nc.scalar.copy(out=t2row[:, 41:42], in_=ps2[:, 40:41])
                        ad2t = pw.tile([128, 1], f16, tag="ad2t")
                        nc.scalar.copy(out=ad2t[:], in_=ps2[:, 41:42])
                        nc.sync.dma_start(
                            tab2in[w * 128:(w + 1) * 128, 0:42], t2row[:])
                        nc.sync.dma_start(
                            taba[w * 128:(w + 1) * 128, 8:9], ad2t[:])

            # ---------- AllGather layer-2 table ----------
            nc.gpsimd.collective_compute(
                "AllGather", OP.bypass, ins=[tab2in[:]], outs=[tab2[:]],
                replica_groups=[list(range(8))])

            # ---------- phase D: layer-2 edge pass -> out ----------
            with (tc.tile_pool(name="pat2", bufs=1) as pat2,
                  tc.tile_pool(name="pd", bufs=2) as pd,
                  tc.tile_pool(name="pw2", bufs=3) as pw2,
                  tc.tile_pool(name="ppd", bufs=2, space="PSUM") as ppd):
                at2 = pat2.tile([128, NSLOT // 128, 1], f16, tag="at2")
                _dma_gather_small(nc.gpsimd, at2[:], taba[:, 8:9],
                                  adix_sb[:], NSLOT, 1, 128, queue_num=0)

                for b in range(NBLK):
                    hix = pd.tile([128, 1600], i16, tag="hix2")
                    nc.sync.dma_start(hix[:],
                                      hidx[:, b * 1600:(b + 1) * 1600])
                    g2 = pd.tile([128, NQ, B * 5, 42], f16, tag="g2")
                    for qi in range(NQ):
                        _dma_gather_small(
                            nc.gpsimd, g2[:, qi], tab2[qi * QS:(qi + 1) * QS, 0:42],
                            hix[:, qi * 400:(qi + 1) * 400],
                            B * SQ, 42, 128, queue_num=qi)

                    for wi in range(B):
                        w = b * B + wi
                        oh = pw2.tile([128, 128, NQ, 5], bf16, tag="oh2")
                        dr = drel_sb[:, b * 200 + wi * 5:(b + 1) * 200]
                        nc.vector.tensor_tensor(
                            out=oh[:],
                            in0=BC(dr, [dr.ap[0], [0, 128], [50, NQ], [1, 5]]),
                            in1=BC(iotaT5[:],
                                   [iotaT5[:].ap[0], [5, 128], [0, NQ], [1, 5]]),
                            op=OP.is_equal)

                        e2 = pw2.tile([128, NQ, 5], f32, tag="e2")
                        gsl = g2[:, :, wi * 5:(wi + 1) * 5, 41:42]
                        asl = at2[:, b * 200 + wi * 5:, :]
                        nc.vector.tensor_tensor(
                            out=e2[:],
                            in0=BC(gsl, [gsl.ap[0], [2100, NQ], [42, 5]]),
                            in1=BC(asl, [asl.ap[0], [50, NQ], [1, 5]]),
                            op=OP.add)
                        lr2 = pw2.tile([128, NQ, 5], f32, tag="lr2")
                        nc.scalar.activation(out=lr2[:], in_=e2[:],
                                             func=AF.Prelu, alpha=0.2)
                        # 41-expanded exp on ACT so the m2 mult runs packed 2x
                        wgx2 = pw2.tile([128, NQ, 5, 41], bf16, tag="wgx2")
                        l2f = lr2[:]
                        nc.scalar.activation(
                            out=wgx2[:],
                            in_=BC(l2f, [l2f.ap[0], [5, NQ], [1, 5], [0, 41]]),
                            func=AF.Exp)

                        m2 = pw2.tile([128, NQ, 5, 41], bf16, tag="m2")
                        nc.vector.tensor_tensor(
                            out=m2[:], in0=g2[:, :, wi * 5:(wi + 1) * 5, 0:41],
                            in1=wgx2[:], op=OP.mult)

                        ps = ppd.tile([128, 41], f32, tag="agg2")
                        for k in range(NQ * 5):
                            qi, j = divmod(k, 5)
                            nc.tensor.matmul(ps[:], lhsT=oh[:, :, qi, j],
                                             rhs=m2[:, qi, j, :],
                                             start=(k == 0),
                                             stop=(k == NQ * 5 - 1))

                        den2 = pw2.tile([128, 1], f32, tag="den2")
                        nc.vector.tensor_scalar_add(den2[:], ps[:, 40:41],
                                                    ACC_EPS)
                        rec2 = pw2.tile([128, 1], f32, tag="rec2")
                        nc.vector.reciprocal(rec2[:], den2[:])
                        o2 = pw2.tile([128, 40], f32, tag="o2")
                        nc.vector.tensor_scalar_mul(o2[:], ps[:, 0:40], rec2[:])
                        o2b = pw2.tile([128, 40], f32, tag="o2b")
                        nc.vector.tensor_tensor(out=o2b[:], in0=o2[:],
                                                in1=b2r[:], op=OP.add)
                        nc.sync.dma_start(out[w * 128:(w + 1) * 128, :], o2b[:])

    nc.finalize()
    return nc


def kernel(**inputs):
    per_core = _host_prep(**inputs)
    if "nc" not in _CACHE:
        _CACHE["nc"] = _build_nc()
    nc = _CACHE["nc"]
    from concourse.bass_utils import run_bass_kernel_spmd
    res = run_bass_kernel_spmd(nc, per_core, list(range(8)))
    full = np.concatenate([res.results[c]["out"] for c in range(8)], axis=0)
    return np.ascontiguousarray(full[:N]).astype(np.float32)
